# revision 10
# baseline (speedup 1.0000x reference)
"""Distributed Trainium2 Bass kernel for causal multi-head attention (RoPE).

Reference computation (B=2, S=2048, D=2048, H=16, hd=128):
    q/k/v = x @ w{q,k,v}.T ; rope(q, k) ; causal softmax attention ; out @ wo.T

Sharding over 8 NeuronCores (tensor-parallel over heads, then rows):
  - Each core owns 2 heads: computes its q/k/v projections (256 features),
    RoPE, and causal attention for those heads.
  - Attention outputs (pre-normalized by softmax denominator via a broadcast
    trick) are exchanged with a single AllToAll so each core ends up with
    ALL features for 1/8 of the token rows.
  - Each core computes its 512 rows of the output projection; the host
    concatenates the 8 row-chunks.

Everything is computed in bf16 on the TensorEngine with f32 PSUM
accumulation; softmax runs without max-subtraction (scores are O(1) by
construction) with the causal mask applied as a 0/1 multiply after exp.

Layout tricks:
  - Activations live feature-major (xT, qT, kT) so matmul contractions are
    natural; v is produced token-major directly by swapping matmul operands.
  - Scores are computed transposed (sT[j, i]) so no on-chip transposes of the
    softmax matrix are needed; softmax sums over partitions use a ones-vector
    matmul; the per-token 1/Z is broadcast across partitions with a K=1
    matmul.
  - RoPE pair-swap (partition crossing) is done with a permutation-matrix
    matmul; cos/sin tables are pre-expanded on the host.
"""

import numpy as np
import ml_dtypes

import concourse.mybir as mybir
import concourse.tile as tile
from concourse import bacc
from concourse.bass_utils import run_bass_kernel_spmd

# Problem constants (hardcoded per harness contract)
B, S, D, H = 2, 2048, 2048, 16
W = 8  # cores
N = B * S  # 4096 tokens
HD = D // H  # 128 head dim
HL = H // W  # 2 heads per core
DL = HL * HD  # 256 features per core
CH = 512  # token chunk
NCH = N // CH  # 8 chunks
KT = D // 128  # 16 contraction tiles
RPC = N // W  # 512 rows per core for the output projection
NVB = N // 128  # 32 v token-blocks
SB = S // CH  # 4 i-chunks per batch

F32 = mybir.dt.float32
BF16 = mybir.dt.bfloat16
MUL = mybir.AluOpType.mult
ADD = mybir.AluOpType.add


def build_nc(dumps=False):
    nc = bacc.Bacc("TRN2", target_bir_lowering=False, debug=False, num_devices=W)

    xT = nc.dram_tensor("xT", [D, N], BF16, kind="ExternalInput").ap()
    wqT = nc.dram_tensor("wqT", [D, DL], BF16, kind="ExternalInput").ap()
    wkT = nc.dram_tensor("wkT", [D, DL], BF16, kind="ExternalInput").ap()
    wvT = nc.dram_tensor("wvT", [D, DL], BF16, kind="ExternalInput").ap()
    woT = nc.dram_tensor("woT", [D, D], BF16, kind="ExternalInput").ap()
    fc2 = nc.dram_tensor("fc2", [HD, N], F32, kind="ExternalInput").ap()
    fss = nc.dram_tensor("fss", [HD, N], F32, kind="ExternalInput").ap()
    pswap = nc.dram_tensor("pswap", [HD, HD], BF16, kind="ExternalInput").ap()
    mask01 = nc.dram_tensor("mask01", [128, 4, CH], BF16, kind="ExternalInput").ap()
    out = nc.dram_tensor("out", [RPC, D], F32, kind="ExternalOutput").ap()

    dbg = None
    if dumps:
        dbg = {
            "dbg_q": nc.dram_tensor("dbg_q", [128, HL, N], BF16, kind="ExternalOutput").ap(),
            "dbg_k": nc.dram_tensor("dbg_k", [128, HL, N], BF16, kind="ExternalOutput").ap(),
            "dbg_v": nc.dram_tensor("dbg_v", [128, NVB, DL], BF16, kind="ExternalOutput").ap(),
            "dbg_a2a": nc.dram_tensor("dbg_a2a", [W, DL, CH], BF16, kind="ExternalOutput").ap(),
            "dbg_attn": nc.dram_tensor("dbg_attn", [128, KT, CH], BF16, kind="ExternalOutput").ap(),
        }

    with tile.TileContext(nc) as tc:
        _body(tc, xT, wqT, wkT, wvT, woT, fc2, fss, pswap, mask01, out, dbg)

    nc.compile()
    return nc


def _body(tc, xT, wqT, wkT, wvT, woT, fc2, fss, pswap, mask01, out, dbg=None):
    nc = tc.nc
    EXP = mybir.ActivationFunctionType.Exp

    with (
        tc.tile_pool(name="const", bufs=1) as const,
        tc.tile_pool(name="dram", bufs=1, space="DRAM") as dram,
    ):
        # ---- persistent SBUF state ----
        wq_sb = const.tile([128, KT, DL], BF16)
        wk_sb = const.tile([128, KT, DL], BF16)
        wv_sb = const.tile([128, KT, DL], BF16)
        nc.sync.dma_start(wq_sb[:], wqT.rearrange("(kt p) m -> p kt m", p=128))
        nc.sync.dma_start(wk_sb[:], wkT.rearrange("(kt p) m -> p kt m", p=128))
        nc.sync.dma_start(wv_sb[:], wvT.rearrange("(kt p) m -> p kt m", p=128))
        fc2_sb = const.tile([128, N], F32)
        fss_sb = const.tile([128, N], F32)
        nc.sync.dma_start(fc2_sb[:], fc2)
        nc.sync.dma_start(fss_sb[:], fss)
        pswap_sb = const.tile([128, 128], BF16)
        nc.sync.dma_start(pswap_sb[:], pswap)
        mask_sb = const.tile([128, 4, CH], BF16)
        nc.sync.dma_start(mask_sb[:], mask01)
        ones_col = const.tile([128, 1], BF16)
        nc.vector.memset(ones_col[:], 1.0)
        ones_row = const.tile([1, 128], BF16)
        nc.vector.memset(ones_row[:], 1.0)

        qT_sb = const.tile([128, HL, N], BF16)  # feature-major q (post-rope)
        kT_sb = const.tile([128, HL, N], BF16)
        v_sb = const.tile([128, NVB, DL], BF16)  # token-major v
        attn_sb = const.tile([128, KT, CH], BF16)  # post-A2A rows, feature-major

        a2a_in = dram.tile([W, DL, CH], BF16)
        a2a_out = dram.tile([W, DL, CH], BF16)

        # ================= stage 1: q/k/v projections + RoPE =================
        with (
            tc.tile_pool(name="xin", bufs=4) as xin_pool,
            tc.tile_pool(name="ev", bufs=4) as ev_pool,
            tc.tile_pool(name="ps1", bufs=2, space="PSUM") as ps1,
        ):
            for ch in range(NCH):
                tok = slice(ch * CH, (ch + 1) * CH)
                ps_q = [
                    ps1.tile([128, CH], F32, tag=f"pq{s}", name=f"ps_q{s}", bufs=1)
                    for s in range(2)
                ]
                ps_k = [
                    ps1.tile([128, CH], F32, tag=f"pk{s}", name=f"ps_k{s}", bufs=1)
                    for s in range(2)
                ]
                ps_v = [
                    ps1.tile([128, 2, 256], F32, tag=f"pv{s}", name=f"ps_v{s}", bufs=1)
                    for s in range(2)
                ]
                for kt in range(KT):
                    xt = xin_pool.tile([128, CH], BF16, tag="xt")
                    nc.sync.dma_start(
                        xt[:], xT[kt * 128 : (kt + 1) * 128, tok]
                    )
                    st, sp = kt == 0, kt == KT - 1
                    for sub in range(2):
                        fsl = slice(sub * 128, (sub + 1) * 128)
                        nc.tensor.matmul(
                            ps_q[sub][:], wq_sb[:, kt, fsl], xt[:], start=st, stop=sp
                        )
                        nc.tensor.matmul(
                            ps_k[sub][:], wk_sb[:, kt, fsl], xt[:], start=st, stop=sp
                        )
                    for t in range(4):
                        # start=True zeroes the whole 2KB PSUM bank, so only
                        # the bank's first slice may set it (kt==0, even t)
                        nc.tensor.matmul(
                            ps_v[t // 2][:, t % 2, :],
                            xt[:, t * 128 : (t + 1) * 128],
                            wv_sb[:, kt, :],
                            start=(st and t % 2 == 0),
                            stop=sp,
                        )
                # evict v (token-major)
                for half in range(2):
                    nc.vector.tensor_copy(
                        v_sb[:, ch * 4 + half * 2 : ch * 4 + half * 2 + 2, :],
                        ps_v[half][:],
                    )
                # RoPE: q' = q*cos2 + swap(q)*sgn_sin2
                for ps_pair, dst in ((ps_q, qT_sb), (ps_k, kT_sb)):
                    for sub in range(2):
                        tmp = ev_pool.tile([128, CH], BF16, tag="tmp")
                        nc.vector.tensor_copy(tmp[:], ps_pair[sub][:])
                        ps_sw = ps1.tile([128, CH], F32, tag="psw")
                        nc.tensor.matmul(
                            ps_sw[:], pswap_sb[:], tmp[:], start=True, stop=True
                        )
                        t1 = ev_pool.tile([128, CH], F32, tag="t1")
                        t2 = ev_pool.tile([128, CH], F32, tag="t2")
                        nc.vector.tensor_tensor(
                            t1[:], ps_pair[sub][:], fc2_sb[:, tok], MUL
                        )
                        nc.vector.tensor_tensor(t2[:], ps_sw[:], fss_sb[:, tok], MUL)
                        nc.vector.tensor_tensor(dst[:, sub, tok], t1[:], t2[:], ADD)

        # ================= stage 2: causal attention per (batch, head) =======
        with (
            tc.tile_pool(name="pt", bufs=4) as pt_pool,
            tc.tile_pool(name="zv", bufs=2) as zv_pool,
            tc.tile_pool(name="ot", bufs=3) as ot_pool,
            tc.tile_pool(name="ps2", bufs=2, space="PSUM") as ps2,
        ):
            for b in range(B):
                for h in range(HL):
                    for ci in range(SB):
                        tok_i = slice(b * S + ci * CH, b * S + (ci + 1) * CH)
                        ps_o = ps2.tile([128, CH], F32, tag="po")
                        zv = zv_pool.tile([128, CH], F32, tag="zv")
                        njb = 4 * ci + 4
                        for jb in range(njb):
                            tok_j = slice(b * S + jb * 128, b * S + (jb + 1) * 128)
                            ps_s = ps2.tile([128, CH], F32, tag="ps")
                            nc.tensor.matmul(
                                ps_s[:],
                                kT_sb[:, h, tok_j],
                                qT_sb[:, h, tok_i],
                                start=True,
                                stop=True,
                            )
                            pt = pt_pool.tile([128, CH], BF16, tag="pt")
                            nc.scalar.activation(pt[:], ps_s[:], EXP)
                            if jb >= 4 * ci:
                                nc.vector.tensor_tensor(
                                    pt[:], pt[:], mask_sb[:, jb - 4 * ci, :], MUL
                                )
                            if jb == 0:
                                nc.vector.tensor_copy(zv[:], pt[:])
                            else:
                                nc.vector.tensor_tensor(zv[:], zv[:], pt[:], ADD)
                            vb = b * (S // 128) + jb
                            nc.tensor.matmul(
                                ps_o[:],
                                v_sb[:, vb, h * 128 : (h + 1) * 128],
                                pt[:],
                                start=(jb == 0),
                                stop=(jb == njb - 1),
                            )
                        # normalize by 1/Z (partition-sum via ones matmul,
                        # partition-broadcast via K=1 matmul)
                        zvb = pt_pool.tile([128, CH], BF16, tag="zvb")
                        nc.vector.tensor_copy(zvb[:], zv[:])
                        ps_z = ps2.tile([1, CH], F32, tag="pz")
                        nc.tensor.matmul(
                            ps_z[:], ones_col[:], zvb[:], start=True, stop=True
                        )
                        rz = ot_pool.tile([1, CH], F32, tag="rz")
                        nc.vector.reciprocal(rz[:], ps_z[:])
                        rzb = ot_pool.tile([1, CH], BF16, tag="rzb")
                        nc.vector.tensor_copy(rzb[:], rz[:])
                        ps_bc = ps2.tile([128, CH], F32, tag="pbc")
                        nc.tensor.matmul(
                            ps_bc[:], ones_row[:], rzb[:], start=True, stop=True
                        )
                        bc_sb = ot_pool.tile([128, CH], F32, tag="bc_sb")
                        nc.vector.tensor_copy(bc_sb[:], ps_bc[:])
                        otn = ot_pool.tile([128, CH], BF16, tag="otn")
                        nc.vector.tensor_tensor(otn[:], ps_o[:], bc_sb[:], MUL)
                        sh = b * SB + ci
                        nc.sync.dma_start(
                            a2a_in[sh, h * 128 : (h + 1) * 128, :], otn[:]
                        )

        # ================= stage 3: AllToAll (head-shard -> row-shard) =======
        nc.gpsimd.collective_compute(
            "AllToAll",
            mybir.AluOpType.bypass,
            replica_groups=[list(range(W))],
            ins=[a2a_in.opt()],
            outs=[a2a_out.opt()],
        )

        # ================= stage 4: output projection for this core's rows ===
        a2a_flat = a2a_out[:].rearrange("w d c -> (w d) c")  # [2048, 512]
        with (
            tc.tile_pool(name="wo", bufs=32) as wo_pool,
            tc.tile_pool(name="oev", bufs=3) as oev_pool,
            tc.tile_pool(name="ps4", bufs=4, space="PSUM") as ps4,
        ):
            for kt in range(KT):
                nc.sync.dma_start(
                    attn_sb[:, kt, :], a2a_flat[kt * 128 : (kt + 1) * 128, :]
                )
            for n in range(D // CH):
                wts = []
                for kt in range(KT):
                    wt = wo_pool.tile([128, CH], BF16, tag="wo")
                    nc.sync.dma_start(
                        wt[:],
                        woT[kt * 128 : (kt + 1) * 128, n * CH : (n + 1) * CH],
                    )
                    wts.append(wt)
                for m in range(RPC // 128):
                    ps_out = ps4.tile([128, CH], F32, tag="pout")
                    for kt in range(KT):
                        nc.tensor.matmul(
                            ps_out[:],
                            attn_sb[:, kt, m * 128 : (m + 1) * 128],
                            wts[kt][:],
                            start=(kt == 0),
                            stop=(kt == KT - 1),
                        )
                    oev = oev_pool.tile([128, CH], F32, tag="oev")
                    nc.vector.tensor_copy(oev[:], ps_out[:])
                    nc.sync.dma_start(
                        out[m * 128 : (m + 1) * 128, n * CH : (n + 1) * CH], oev[:]
                    )

        if dbg is not None:
            nc.sync.dma_start(dbg["dbg_q"], qT_sb[:])
            nc.sync.dma_start(dbg["dbg_k"], kT_sb[:])
            nc.sync.dma_start(dbg["dbg_v"], v_sb[:])
            nc.sync.dma_start(dbg["dbg_a2a"], a2a_in[:])
            nc.sync.dma_start(dbg["dbg_attn"], attn_sb[:])


_NC_CACHE = None


def _get_nc():
    global _NC_CACHE
    if _NC_CACHE is None:
        _NC_CACHE = build_nc()
    return _NC_CACHE


def _prep_inputs(x, freq_cos, freq_sin, wq, wk, wv, wo):
    bf = ml_dtypes.bfloat16
    x = np.asarray(x, np.float32).reshape(N, D)
    fc = np.asarray(freq_cos, np.float32)  # [S, 64]
    fs = np.asarray(freq_sin, np.float32)
    wq = np.asarray(wq, np.float32)
    wk = np.asarray(wk, np.float32)
    wv = np.asarray(wv, np.float32)
    wo = np.asarray(wo, np.float32)

    xT = np.ascontiguousarray(x.T).astype(bf)  # [D, N]
    woT = np.ascontiguousarray(wo.T).astype(bf)  # [D, D]

    # RoPE tables, expanded to the full head dim and tiled over batch.
    # fc2[d, b*S+i] = cos(freq[i, d//2]); fss carries sin with the sign of the
    # pair-swap term: -sin for even d, +sin for odd d.
    fc2 = np.tile(np.repeat(fc.T, 2, axis=0), (1, B)).astype(np.float32)
    sgn = np.where(np.arange(HD) % 2 == 0, -1.0, 1.0).astype(np.float32)[:, None]
    fss = (np.tile(np.repeat(fs.T, 2, axis=0), (1, B)) * sgn).astype(np.float32)
    fc2 = np.ascontiguousarray(fc2)
    fss = np.ascontiguousarray(fss)

    pswap = np.zeros((HD, HD), np.float32)
    pswap[np.arange(HD) ^ 1, np.arange(HD)] = 1.0
    pswap = pswap.astype(bf)

    # causal 0/1 mask for the 4 diagonal j-block positions of each 512 i-chunk
    jp = np.arange(128)[:, None, None]
    r = np.arange(4)[None, :, None]
    if_ = np.arange(CH)[None, None, :]
    mask01 = ((128 * r + jp) <= if_).astype(np.float32).astype(bf)

    scale = 1.0 / np.sqrt(HD)
    in_maps = []
    for c in range(W):
        rows = slice(c * DL, (c + 1) * DL)
        wqT = np.ascontiguousarray((wq[rows] * scale).T).astype(bf)
        wkT = np.ascontiguousarray(wk[rows].T).astype(bf)
        wvT = np.ascontiguousarray(wv[rows].T).astype(bf)
        in_maps.append(
            {
                "xT": xT,
                "wqT": wqT,
                "wkT": wkT,
                "wvT": wvT,
                "woT": woT,
                "fc2": fc2,
                "fss": fss,
                "pswap": pswap,
                "mask01": mask01,
            }
        )
    return in_maps


def kernel(x, freq_cos, freq_sin, wq, wk, wv, wo, _trace=False, _trace_kwargs=None):
    nc = _get_nc()
    in_maps = _prep_inputs(x, freq_cos, freq_sin, wq, wk, wv, wo)
    kwargs = {}
    if _trace:
        kwargs.update(trace=True, **(_trace_kwargs or {}))
    res = run_bass_kernel_spmd(nc, in_maps, core_ids=list(range(W)), **kwargs)
    kernel.last_result = res
    full = np.concatenate([res.results[c]["out"] for c in range(W)], axis=0)
    return full.reshape(B, S, D).astype(np.float32)


# revision 15
# speedup vs baseline: 1.2246x; 1.2246x over previous
"""Distributed Trainium2 Bass kernel for causal multi-head attention (RoPE).

Reference computation (B=2, S=2048, D=2048, H=16, hd=128):
    q/k/v = x @ w{q,k,v}.T ; rope(q, k) ; causal softmax attention ; out @ wo.T

Sharding over 8 NeuronCores (tensor-parallel over heads, then rows):
  - Each core owns 2 heads: computes its q/k/v projections (256 features),
    RoPE, and causal attention for those heads.
  - Attention outputs (normalized by the softmax denominator via a broadcast
    trick) are exchanged with one AllToAll per local head so each core ends
    up with ALL features for 1/8 of the token rows; the per-head split lets
    the first collective overlap the second head's attention compute.
  - Each core computes its 512 rows of the output projection; the host
    concatenates the 8 row-chunks.

Everything is computed in bf16 on the TensorEngine with f32 PSUM
accumulation; softmax runs without max-subtraction (scores are O(1) by
construction) with the causal mask applied as a 0/1 multiply after exp.

Layout tricks:
  - Activations live feature-major (xT, qT, kT) so matmul contractions are
    natural; v is produced token-major directly by swapping matmul operands.
  - Scores are computed transposed (sT[j, i]) so no on-chip transposes of the
    softmax matrix are needed; softmax sums over partitions use a ones-vector
    matmul; the per-token 1/Z is broadcast across partitions with a K=1
    matmul.
  - RoPE pair-swap (partition crossing) is done with a permutation-matrix
    matmul; cos/sin tables are pre-expanded on the host.
  - Causal structure: fully-masked j-blocks are skipped; on the 4 diagonal
    j-blocks of each 512-wide i-chunk only the live suffix of queries is
    computed, so just one triangular 128x128 corner needs the 0/1 mask.
"""

import numpy as np
import ml_dtypes

import concourse.mybir as mybir
import concourse.tile as tile
from concourse import bacc
from concourse.bass_utils import run_bass_kernel_spmd

# Problem constants (hardcoded per harness contract)
B, S, D, H = 2, 2048, 2048, 16
W = 8  # cores
N = B * S  # 4096 tokens
HD = D // H  # 128 head dim
HL = H // W  # 2 heads per core
DL = HL * HD  # 256 features per core
CH = 512  # token chunk
NCH = N // CH  # 8 chunks
KT = D // 128  # 16 contraction tiles
RPC = N // W  # 512 rows per core for the output projection
NVB = N // 128  # 32 v token-blocks
SB = S // CH  # 4 i-chunks per batch

F32 = mybir.dt.float32
BF16 = mybir.dt.bfloat16
MUL = mybir.AluOpType.mult
ADD = mybir.AluOpType.add


def build_nc(dumps=False):
    nc = bacc.Bacc("TRN2", target_bir_lowering=False, debug=False, num_devices=W)

    xT = nc.dram_tensor("xT", [D, N], BF16, kind="ExternalInput").ap()
    wqT = nc.dram_tensor("wqT", [D, DL], BF16, kind="ExternalInput").ap()
    wkT = nc.dram_tensor("wkT", [D, DL], BF16, kind="ExternalInput").ap()
    wvT = nc.dram_tensor("wvT", [D, DL], BF16, kind="ExternalInput").ap()
    woT = nc.dram_tensor("woT", [D, D], BF16, kind="ExternalInput").ap()
    fc2 = nc.dram_tensor("fc2", [HD, N], F32, kind="ExternalInput").ap()
    fss = nc.dram_tensor("fss", [HD, N], F32, kind="ExternalInput").ap()
    pswap = nc.dram_tensor("pswap", [HD, HD], BF16, kind="ExternalInput").ap()
    mask01 = nc.dram_tensor("mask01", [128, 128], BF16, kind="ExternalInput").ap()
    out = nc.dram_tensor("out", [RPC, D], F32, kind="ExternalOutput").ap()

    dbg = None
    if dumps:
        dbg = {
            "dbg_q": nc.dram_tensor("dbg_q", [128, HL, N], BF16, kind="ExternalOutput").ap(),
            "dbg_k": nc.dram_tensor("dbg_k", [128, HL, N], BF16, kind="ExternalOutput").ap(),
            "dbg_v": nc.dram_tensor("dbg_v", [128, NVB, DL], BF16, kind="ExternalOutput").ap(),
            "dbg_attn": nc.dram_tensor("dbg_attn", [128, KT, CH], BF16, kind="ExternalOutput").ap(),
        }

    with tile.TileContext(nc) as tc:
        _body(tc, xT, wqT, wkT, wvT, woT, fc2, fss, pswap, mask01, out, dbg)

    nc.compile()
    return nc


def _body(tc, xT, wqT, wkT, wvT, woT, fc2, fss, pswap, mask01, out, dbg=None):
    nc = tc.nc
    EXP = mybir.ActivationFunctionType.Exp

    with (
        tc.tile_pool(name="const", bufs=1) as const,
        tc.tile_pool(name="dram", bufs=1, space="DRAM") as dram,
    ):
        # ---- persistent SBUF state (weights first: stage 1 needs them now) ----
        wq_sb = const.tile([128, KT, DL], BF16)
        wk_sb = const.tile([128, KT, DL], BF16)
        wv_sb = const.tile([128, KT, DL], BF16)
        nc.sync.dma_start(wq_sb[:], wqT.rearrange("(kt p) m -> p kt m", p=128))
        nc.sync.dma_start(wk_sb[:], wkT.rearrange("(kt p) m -> p kt m", p=128))
        nc.sync.dma_start(wv_sb[:], wvT.rearrange("(kt p) m -> p kt m", p=128))
        pswap_sb = const.tile([128, 128], BF16)
        nc.sync.dma_start(pswap_sb[:], pswap)
        mask_sb = const.tile([128, 128], BF16)
        ones_col = const.tile([128, 1], BF16)
        nc.vector.memset(ones_col[:], 1.0)
        ones_row = const.tile([1, 128], BF16)
        nc.vector.memset(ones_row[:], 1.0)

        qT_sb = const.tile([128, HL, N], BF16)  # feature-major q (post-rope)
        kT_sb = const.tile([128, HL, N], BF16)
        v_sb = const.tile([128, NVB, DL], BF16)  # token-major v
        # post-A2A row tiles, feature-major; one tile per k-tile so phase-A
        # matmuls only depend on the first AllToAll's DMAs
        attn_t = [
            const.tile([128, CH], BF16, name=f"attn_t{kt}") for kt in range(KT)
        ]

        # per-head A2A buffers (shard s of head h = oT for rows [512s, 512s+512))
        a2a_in = [dram.tile([W, HD, CH], BF16, name=f"a2a_in{h}") for h in range(HL)]
        a2a_out = [dram.tile([W, HD, CH], BF16, name=f"a2a_out{h}") for h in range(HL)]

        # ================= stage 1: q/k/v projections + RoPE =================
        with (
            tc.tile_pool(name="xin", bufs=4) as xin_pool,
            tc.tile_pool(name="ev", bufs=4) as ev_pool,
            tc.tile_pool(name="frq", bufs=1) as frq_pool,
            tc.tile_pool(name="ps1", bufs=2, space="PSUM") as ps1,
        ):
            fc2_sb = frq_pool.tile([128, N], F32)
            fss_sb = frq_pool.tile([128, N], F32)
            for ch in range(NCH):
                tok = slice(ch * CH, (ch + 1) * CH)
                ps_q = [
                    ps1.tile([128, CH], F32, tag=f"pq{s}", name=f"ps_q{s}", bufs=1)
                    for s in range(2)
                ]
                ps_k = [
                    ps1.tile([128, CH], F32, tag=f"pk{s}", name=f"ps_k{s}", bufs=1)
                    for s in range(2)
                ]
                ps_v = [
                    ps1.tile([128, 2, 256], F32, tag=f"pv{s}", name=f"ps_v{s}", bufs=1)
                    for s in range(2)
                ]
                for kt in range(KT):
                    xt = xin_pool.tile([128, CH], BF16, tag="xt")
                    nc.sync.dma_start(xt[:], xT[kt * 128 : (kt + 1) * 128, tok])
                    st, sp = kt == 0, kt == KT - 1
                    for sub in range(2):
                        fsl = slice(sub * 128, (sub + 1) * 128)
                        nc.tensor.matmul(
                            ps_q[sub][:], wq_sb[:, kt, fsl], xt[:], start=st, stop=sp
                        )
                        nc.tensor.matmul(
                            ps_k[sub][:], wk_sb[:, kt, fsl], xt[:], start=st, stop=sp
                        )
                    for t in range(4):
                        # start=True zeroes the whole 2KB PSUM bank, so only
                        # the bank's first slice may set it (kt==0, even t)
                        nc.tensor.matmul(
                            ps_v[t // 2][:, t % 2, :],
                            xt[:, t * 128 : (t + 1) * 128],
                            wv_sb[:, kt, :],
                            start=(st and t % 2 == 0),
                            stop=sp,
                        )
                if ch == 0:
                    # issued here (after chunk-0 loads) so they don't delay the
                    # first matmuls; only needed by the rope eviction below
                    nc.sync.dma_start(fc2_sb[:], fc2)
                    nc.sync.dma_start(fss_sb[:], fss)
                    nc.sync.dma_start(mask_sb[:], mask01)
                # evict v (token-major)
                for half in range(2):
                    nc.vector.tensor_copy(
                        v_sb[:, ch * 4 + half * 2 : ch * 4 + half * 2 + 2, :],
                        ps_v[half][:],
                    )
                # RoPE: q' = q*cos2 + swap(q)*sgn_sin2
                for ps_pair, dst in ((ps_q, qT_sb), (ps_k, kT_sb)):
                    for sub in range(2):
                        tmp = ev_pool.tile([128, CH], BF16, tag="tmp")
                        nc.vector.tensor_copy(tmp[:], ps_pair[sub][:])
                        ps_sw = ps1.tile([128, CH], F32, tag="psw")
                        nc.tensor.matmul(
                            ps_sw[:], pswap_sb[:], tmp[:], start=True, stop=True
                        )
                        t1 = ev_pool.tile([128, CH], F32, tag="t1")
                        t2 = ev_pool.tile([128, CH], F32, tag="t2")
                        nc.vector.tensor_tensor(
                            t1[:], ps_pair[sub][:], fc2_sb[:, tok], MUL
                        )
                        nc.vector.tensor_tensor(t2[:], ps_sw[:], fss_sb[:, tok], MUL)
                        nc.vector.tensor_tensor(dst[:, sub, tok], t1[:], t2[:], ADD)

        # ================= stage 2: causal attention (head-outer) =============
        with (
            tc.tile_pool(name="pt", bufs=4) as pt_pool,
            tc.tile_pool(name="zv", bufs=2) as zv_pool,
            tc.tile_pool(name="ot", bufs=3) as ot_pool,
            tc.tile_pool(name="ps2", bufs=2, space="PSUM") as ps2,
        ):
            for h in range(HL):
                for b in range(B):
                    for ci in range(SB):
                        tok_i0 = b * S + ci * CH
                        ps_o = ps2.tile([128, CH], F32, tag="po")
                        zv = zv_pool.tile([128, CH], F32, tag="zv")
                        njb = 4 * ci + 4
                        for jb in range(njb):
                            tok_j = slice(b * S + jb * 128, b * S + (jb + 1) * 128)
                            r = jb - 4 * ci  # diag position (>=0 on diagonal)
                            off = 128 * r if r > 0 else 0  # live query suffix
                            wid = CH - off
                            ps_s = ps2.tile([128, CH], F32, tag="ps")
                            nc.tensor.matmul(
                                ps_s[:, :wid],
                                kT_sb[:, h, tok_j],
                                qT_sb[:, h, tok_i0 + off : tok_i0 + CH],
                                start=True,
                                stop=True,
                            )
                            pt = pt_pool.tile([128, CH], BF16, tag="pt")
                            nc.scalar.activation(pt[:, :wid], ps_s[:, :wid], EXP)
                            if r >= 0:
                                # triangular corner: queries [128r, 128r+128)
                                nc.vector.tensor_tensor(
                                    pt[:, :128], pt[:, :128], mask_sb[:], MUL
                                )
                            if jb == 0:
                                nc.vector.tensor_copy(zv[:], pt[:])
                            else:
                                nc.vector.tensor_tensor(
                                    zv[:, off:], zv[:, off:], pt[:, :wid], ADD
                                )
                            vb = b * (S // 128) + jb
                            nc.tensor.matmul(
                                ps_o[:, off:],
                                v_sb[:, vb, h * 128 : (h + 1) * 128],
                                pt[:, :wid],
                                start=(jb == 0),
                                stop=(jb == njb - 1),
                            )
                        # normalize by 1/Z (partition-sum via ones matmul,
                        # partition-broadcast via K=1 matmul)
                        zvb = pt_pool.tile([128, CH], BF16, tag="zvb")
                        nc.vector.tensor_copy(zvb[:], zv[:])
                        ps_z = ps2.tile([1, CH], F32, tag="pz")
                        nc.tensor.matmul(
                            ps_z[:], ones_col[:], zvb[:], start=True, stop=True
                        )
                        rz = ot_pool.tile([1, CH], F32, tag="rz")
                        nc.vector.reciprocal_approx_fast(rz[:], ps_z[:])
                        rzb = ot_pool.tile([1, CH], BF16, tag="rzb")
                        nc.vector.tensor_copy(rzb[:], rz[:])
                        ps_bc = ps2.tile([128, CH], F32, tag="pbc")
                        nc.tensor.matmul(
                            ps_bc[:], ones_row[:], rzb[:], start=True, stop=True
                        )
                        bc_sb = ot_pool.tile([128, CH], F32, tag="bc_sb")
                        nc.vector.tensor_copy(bc_sb[:], ps_bc[:])
                        otn = ot_pool.tile([128, CH], BF16, tag="otn")
                        nc.vector.tensor_tensor(otn[:], ps_o[:], bc_sb[:], MUL)
                        sh = b * SB + ci
                        nc.sync.dma_start(a2a_in[h][sh, :, :], otn[:])

                # ---- per-head AllToAll: head 0's collective overlaps head 1's
                # attention compute; head 1's overlaps the output projection
                nc.gpsimd.collective_compute(
                    "AllToAll",
                    mybir.AluOpType.bypass,
                    replica_groups=[list(range(W))],
                    ins=[a2a_in[h].opt()],
                    outs=[a2a_out[h].opt()],
                )

        # ================= stage 4: output projection for this core's rows ===
        # k-tile 2i+h comes from a2a_out[h] block i (global features of head
        # pair i, local head h)
        # Two phases so ALL even-k (head-0) matmuls can run while the second
        # AllToAll is still in flight: phase A accumulates even k-tiles for
        # every (n, m) output group and parks the partial sums in SBUF; phase
        # B adds the odd k-tiles and writes out.
        with (
            tc.tile_pool(name="wo", bufs=48) as wo_pool,
            tc.tile_pool(name="par", bufs=16) as par_pool,
            tc.tile_pool(name="oev", bufs=3) as oev_pool,
            tc.tile_pool(name="ps4", bufs=4, space="PSUM") as ps4,
        ):
            for kt in range(KT):
                src = a2a_out[kt % 2][:].rearrange("w d c -> (w d) c")
                blk = kt // 2
                nc.sync.dma_start(
                    attn_t[kt][:], src[blk * 128 : (blk + 1) * 128, :]
                )
            NCHUNK = D // CH
            MS = RPC // 128
            wts = {}
            for n in range(NCHUNK):
                for kt in range(0, KT, 2):
                    wt = wo_pool.tile([128, CH], BF16, tag="wo", name=f"wt{n}_{kt}")
                    nc.sync.dma_start(
                        wt[:], woT[kt * 128 : (kt + 1) * 128, n * CH : (n + 1) * CH]
                    )
                    wts[(n, kt)] = wt
            partial = {}
            for n in range(NCHUNK):
                for m in range(MS):
                    ps_out = ps4.tile([128, CH], F32, tag="pout", name="ps_outA")
                    for i, kt in enumerate(range(0, KT, 2)):
                        nc.tensor.matmul(
                            ps_out[:],
                            attn_t[kt][:, m * 128 : (m + 1) * 128],
                            wts[(n, kt)][:],
                            start=(i == 0),
                            stop=(i == KT // 2 - 1),
                        )
                    par = par_pool.tile([128, CH], F32, tag="par", name=f"par{n}_{m}")
                    nc.vector.tensor_copy(par[:], ps_out[:])
                    partial[(n, m)] = par
            for n in range(NCHUNK):
                for kt in range(1, KT, 2):
                    wt = wo_pool.tile([128, CH], BF16, tag="wo", name=f"wt{n}_{kt}")
                    nc.sync.dma_start(
                        wt[:], woT[kt * 128 : (kt + 1) * 128, n * CH : (n + 1) * CH]
                    )
                    wts[(n, kt)] = wt
            for n in range(NCHUNK):
                for m in range(MS):
                    ps_out = ps4.tile([128, CH], F32, tag="pout", name="ps_outB")
                    for i, kt in enumerate(range(1, KT, 2)):
                        nc.tensor.matmul(
                            ps_out[:],
                            attn_t[kt][:, m * 128 : (m + 1) * 128],
                            wts[(n, kt)][:],
                            start=(i == 0),
                            stop=(i == KT // 2 - 1),
                        )
                    oev = oev_pool.tile([128, CH], F32, tag="oev")
                    nc.vector.tensor_tensor(
                        oev[:], ps_out[:], partial[(n, m)][:], ADD
                    )
                    nc.sync.dma_start(
                        out[m * 128 : (m + 1) * 128, n * CH : (n + 1) * CH], oev[:]
                    )

        if dbg is not None:
            nc.sync.dma_start(dbg["dbg_q"], qT_sb[:])
            nc.sync.dma_start(dbg["dbg_k"], kT_sb[:])
            nc.sync.dma_start(dbg["dbg_v"], v_sb[:])
            for kt in range(KT):
                nc.sync.dma_start(dbg["dbg_attn"][:, kt, :], attn_t[kt][:])


_NC_CACHE = None


def _get_nc():
    global _NC_CACHE
    if _NC_CACHE is None:
        _NC_CACHE = build_nc()
    return _NC_CACHE


def _prep_inputs(x, freq_cos, freq_sin, wq, wk, wv, wo):
    bf = ml_dtypes.bfloat16
    x = np.asarray(x, np.float32).reshape(N, D)
    fc = np.asarray(freq_cos, np.float32)  # [S, 64]
    fs = np.asarray(freq_sin, np.float32)
    wq = np.asarray(wq, np.float32)
    wk = np.asarray(wk, np.float32)
    wv = np.asarray(wv, np.float32)
    wo = np.asarray(wo, np.float32)

    xT = np.ascontiguousarray(x.T).astype(bf)  # [D, N]
    woT = np.ascontiguousarray(wo.T).astype(bf)  # [D, D]

    # RoPE tables, expanded to the full head dim and tiled over batch.
    # fc2[d, b*S+i] = cos(freq[i, d//2]); fss carries sin with the sign of the
    # pair-swap term: -sin for even d, +sin for odd d.
    fc2 = np.tile(np.repeat(fc.T, 2, axis=0), (1, B)).astype(np.float32)
    sgn = np.where(np.arange(HD) % 2 == 0, -1.0, 1.0).astype(np.float32)[:, None]
    fss = (np.tile(np.repeat(fs.T, 2, axis=0), (1, B)) * sgn).astype(np.float32)
    fc2 = np.ascontiguousarray(fc2)
    fss = np.ascontiguousarray(fss)

    pswap = np.zeros((HD, HD), np.float32)
    pswap[np.arange(HD) ^ 1, np.arange(HD)] = 1.0
    pswap = pswap.astype(bf)

    # triangular 0/1 mask for the 128x128 diagonal corner: allow j <= i
    jp = np.arange(128)[:, None]
    ii = np.arange(128)[None, :]
    mask01 = (jp <= ii).astype(np.float32).astype(bf)

    scale = 1.0 / np.sqrt(HD)
    in_maps = []
    for c in range(W):
        rows = slice(c * DL, (c + 1) * DL)
        wqT = np.ascontiguousarray((wq[rows] * scale).T).astype(bf)
        wkT = np.ascontiguousarray(wk[rows].T).astype(bf)
        wvT = np.ascontiguousarray(wv[rows].T).astype(bf)
        in_maps.append(
            {
                "xT": xT,
                "wqT": wqT,
                "wkT": wkT,
                "wvT": wvT,
                "woT": woT,
                "fc2": fc2,
                "fss": fss,
                "pswap": pswap,
                "mask01": mask01,
            }
        )
    return in_maps


def kernel(x, freq_cos, freq_sin, wq, wk, wv, wo, _trace=False, _trace_kwargs=None):
    nc = _get_nc()
    in_maps = _prep_inputs(x, freq_cos, freq_sin, wq, wk, wv, wo)
    kwargs = {}
    if _trace:
        kwargs.update(trace=True, **(_trace_kwargs or {}))
    res = run_bass_kernel_spmd(nc, in_maps, core_ids=list(range(W)), **kwargs)
    kernel.last_result = res
    full = np.concatenate([res.results[c]["out"] for c in range(W)], axis=0)
    return full.reshape(B, S, D).astype(np.float32)


# revision 16
# speedup vs baseline: 1.2444x; 1.0162x over previous
"""Distributed Trainium2 Bass kernel for causal multi-head attention (RoPE).

Reference computation (B=2, S=2048, D=2048, H=16, hd=128):
    q/k/v = x @ w{q,k,v}.T ; rope(q, k) ; causal softmax attention ; out @ wo.T

Sharding over 8 NeuronCores (tensor-parallel over heads, then rows):
  - Each core owns 2 heads: computes its q/k/v projections (256 features),
    RoPE, and causal attention for those heads.
  - Attention outputs (normalized by the softmax denominator via a broadcast
    trick) are exchanged with one AllToAll per local head so each core ends
    up with ALL features for 1/8 of the token rows; the per-head split lets
    the first collective overlap the second head's attention compute.
  - Each core computes its 512 rows of the output projection; the host
    concatenates the 8 row-chunks.

Everything is computed in bf16 on the TensorEngine with f32 PSUM
accumulation; softmax runs without max-subtraction (scores are O(1) by
construction) with the causal mask applied as a 0/1 multiply after exp.

Layout tricks:
  - Activations live feature-major (xT, qT, kT) so matmul contractions are
    natural; v is produced token-major directly by swapping matmul operands.
  - Scores are computed transposed (sT[j, i]) so no on-chip transposes of the
    softmax matrix are needed; softmax sums over partitions use a ones-vector
    matmul; the per-token 1/Z is broadcast across partitions with a K=1
    matmul.
  - RoPE pair-swap (partition crossing) is done with a permutation-matrix
    matmul; cos/sin tables are pre-expanded on the host.
  - Causal structure: fully-masked j-blocks are skipped; on the 4 diagonal
    j-blocks of each 512-wide i-chunk only the live suffix of queries is
    computed, so just one triangular 128x128 corner needs the 0/1 mask.
  - The output projection runs in two phases (even k-tiles, then odd) with
    partial sums parked in SBUF, so a full pass of matmuls is available to
    overlap the second AllToAll.
"""

import numpy as np
import ml_dtypes

import concourse.mybir as mybir
import concourse.tile as tile
from concourse import bacc
from concourse.bass_utils import run_bass_kernel_spmd

# Problem constants (hardcoded per harness contract)
B, S, D, H = 2, 2048, 2048, 16
W = 8  # cores
N = B * S  # 4096 tokens
HD = D // H  # 128 head dim
HL = H // W  # 2 heads per core
DL = HL * HD  # 256 features per core
CH = 512  # token chunk
NCH = N // CH  # 8 chunks
KT = D // 128  # 16 contraction tiles
RPC = N // W  # 512 rows per core for the output projection
NVB = N // 128  # 32 v token-blocks
SB = S // CH  # 4 i-chunks per batch

F32 = mybir.dt.float32
BF16 = mybir.dt.bfloat16
MUL = mybir.AluOpType.mult
ADD = mybir.AluOpType.add


def build_nc(dumps=False):
    nc = bacc.Bacc("TRN2", target_bir_lowering=False, debug=False, num_devices=W)

    xT = nc.dram_tensor("xT", [D, N], BF16, kind="ExternalInput").ap()
    wqT = nc.dram_tensor("wqT", [D, DL], BF16, kind="ExternalInput").ap()
    wkT = nc.dram_tensor("wkT", [D, DL], BF16, kind="ExternalInput").ap()
    wvT = nc.dram_tensor("wvT", [D, DL], BF16, kind="ExternalInput").ap()
    woT = nc.dram_tensor("woT", [D, D], BF16, kind="ExternalInput").ap()
    fc2 = nc.dram_tensor("fc2", [HD, N], F32, kind="ExternalInput").ap()
    fss = nc.dram_tensor("fss", [HD, N], F32, kind="ExternalInput").ap()
    pswap = nc.dram_tensor("pswap", [HD, HD], BF16, kind="ExternalInput").ap()
    mask01 = nc.dram_tensor("mask01", [128, 128], BF16, kind="ExternalInput").ap()
    out = nc.dram_tensor("out", [RPC, D], F32, kind="ExternalOutput").ap()

    dbg = None
    if dumps:
        dbg = {
            "dbg_q": nc.dram_tensor("dbg_q", [128, HL, N], BF16, kind="ExternalOutput").ap(),
            "dbg_k": nc.dram_tensor("dbg_k", [128, HL, N], BF16, kind="ExternalOutput").ap(),
            "dbg_v": nc.dram_tensor("dbg_v", [128, NVB, DL], BF16, kind="ExternalOutput").ap(),
            "dbg_attn": nc.dram_tensor("dbg_attn", [128, KT, CH], BF16, kind="ExternalOutput").ap(),
        }

    with tile.TileContext(nc) as tc:
        _body(tc, xT, wqT, wkT, wvT, woT, fc2, fss, pswap, mask01, out, dbg)

    nc.compile()
    return nc


def _body(tc, xT, wqT, wkT, wvT, woT, fc2, fss, pswap, mask01, out, dbg=None):
    nc = tc.nc
    EXP = mybir.ActivationFunctionType.Exp

    with (
        tc.tile_pool(name="const", bufs=1) as const,
        tc.tile_pool(name="dram", bufs=1, space="DRAM") as dram,
    ):
        # ---- persistent SBUF state; per-kt weight tiles give precise deps so
        # the first matmul starts as soon as one 64KB slice has landed
        wq_t, wk_t, wv_t = [], [], []
        for kt in range(KT):
            for lst, src, nm in ((wq_t, wqT, "q"), (wk_t, wkT, "k"), (wv_t, wvT, "v")):
                t = const.tile([128, DL], BF16, name=f"w{nm}_t{kt}")
                nc.sync.dma_start(t[:], src[kt * 128 : (kt + 1) * 128, :])
                lst.append(t)
        pswap_sb = const.tile([128, 128], BF16)
        nc.sync.dma_start(pswap_sb[:], pswap)
        mask_sb = const.tile([128, 128], BF16)
        ones_col = const.tile([128, 1], BF16)
        nc.vector.memset(ones_col[:], 1.0)
        ones_row = const.tile([1, 128], BF16)
        nc.vector.memset(ones_row[:], 1.0)

        qT_sb = const.tile([128, HL, N], BF16)  # feature-major q (post-rope)
        kT_sb = const.tile([128, HL, N], BF16)
        v_sb = const.tile([128, NVB, DL], BF16)  # token-major v
        # post-A2A row tiles, feature-major; one tile per k-tile so phase-A
        # matmuls only depend on the first AllToAll's DMAs
        attn_t = [
            const.tile([128, CH], BF16, name=f"attn_t{kt}") for kt in range(KT)
        ]

        # per-head A2A buffers (shard s of head h = oT for rows [512s, 512s+512))
        a2a_in = [dram.tile([W, HD, CH], BF16, name=f"a2a_in{h}") for h in range(HL)]
        a2a_out = [dram.tile([W, HD, CH], BF16, name=f"a2a_out{h}") for h in range(HL)]

        # ================= stage 1: q/k/v projections + RoPE =================
        with (
            tc.tile_pool(name="xin", bufs=4) as xin_pool,
            tc.tile_pool(name="ev", bufs=4) as ev_pool,
            tc.tile_pool(name="frq", bufs=1) as frq_pool,
            tc.tile_pool(name="ps1", bufs=2, space="PSUM") as ps1,
        ):
            fc2_sb = frq_pool.tile([128, N], F32)
            fss_sb = frq_pool.tile([128, N], F32)
            for ch in range(NCH):
                tok = slice(ch * CH, (ch + 1) * CH)
                ps_q = [
                    ps1.tile([128, CH], F32, tag=f"pq{s}", name=f"ps_q{s}", bufs=1)
                    for s in range(2)
                ]
                ps_k = [
                    ps1.tile([128, CH], F32, tag=f"pk{s}", name=f"ps_k{s}", bufs=1)
                    for s in range(2)
                ]
                ps_v = [
                    ps1.tile([128, 2, 256], F32, tag=f"pv{s}", name=f"ps_v{s}", bufs=1)
                    for s in range(2)
                ]
                for kt in range(KT):
                    xt = xin_pool.tile([128, CH], BF16, tag="xt")
                    nc.sync.dma_start(xt[:], xT[kt * 128 : (kt + 1) * 128, tok])
                    st, sp = kt == 0, kt == KT - 1
                    for sub in range(2):
                        fsl = slice(sub * 128, (sub + 1) * 128)
                        nc.tensor.matmul(
                            ps_q[sub][:], wq_t[kt][:, fsl], xt[:], start=st, stop=sp
                        )
                        nc.tensor.matmul(
                            ps_k[sub][:], wk_t[kt][:, fsl], xt[:], start=st, stop=sp
                        )
                    for t in range(4):
                        # start=True zeroes the whole 2KB PSUM bank, so only
                        # the bank's first slice may set it (kt==0, even t)
                        nc.tensor.matmul(
                            ps_v[t // 2][:, t % 2, :],
                            xt[:, t * 128 : (t + 1) * 128],
                            wv_t[kt][:],
                            start=(st and t % 2 == 0),
                            stop=sp,
                        )
                if ch == 0:
                    # issued here (after chunk-0 loads) so they don't delay the
                    # first matmuls; only needed by the rope eviction below
                    nc.sync.dma_start(fc2_sb[:], fc2)
                    nc.sync.dma_start(fss_sb[:], fss)
                    nc.sync.dma_start(mask_sb[:], mask01)
                # RoPE: q' = q*cos2 + swap(q)*sgn_sin2.  The bf16 pre-rope
                # eviction runs on the Scalar engine so the DVE only carries
                # the 3 rope multiplies; v eviction also goes to Scalar.
                for ps_pair, dst in ((ps_q, qT_sb), (ps_k, kT_sb)):
                    for sub in range(2):
                        tmp = ev_pool.tile([128, CH], BF16, tag="tmp")
                        nc.scalar.copy(tmp[:], ps_pair[sub][:])
                        ps_sw = ps1.tile([128, CH], F32, tag="psw")
                        nc.tensor.matmul(
                            ps_sw[:], pswap_sb[:], tmp[:], start=True, stop=True
                        )
                        t1 = ev_pool.tile([128, CH], F32, tag="t1")
                        t2 = ev_pool.tile([128, CH], F32, tag="t2")
                        nc.vector.tensor_tensor(
                            t1[:], ps_pair[sub][:], fc2_sb[:, tok], MUL
                        )
                        nc.vector.tensor_tensor(t2[:], ps_sw[:], fss_sb[:, tok], MUL)
                        nc.vector.tensor_tensor(dst[:, sub, tok], t1[:], t2[:], ADD)
                # evict v (token-major)
                for half in range(2):
                    nc.scalar.copy(
                        v_sb[:, ch * 4 + half * 2 : ch * 4 + half * 2 + 2, :],
                        ps_v[half][:],
                    )

        # preload ALL wo tiles on the gpsimd (SWDGE) queue so they stream in
        # during attention without delaying the sync-queue A2A staging writes
        with tc.tile_pool(name="wo", bufs=64) as wo_pool:
            wts = {}
            for n in range(D // CH):
                for kt in range(KT):
                    wt = wo_pool.tile([128, CH], BF16, tag="wo", name=f"wt{n}_{kt}")
                    nc.gpsimd.dma_start(
                        wt[:], woT[kt * 128 : (kt + 1) * 128, n * CH : (n + 1) * CH]
                    )
                    wts[(n, kt)] = wt

            # ================= stage 2: causal attention (head-outer) =========
            with (
                tc.tile_pool(name="pt", bufs=4) as pt_pool,
                tc.tile_pool(name="zv", bufs=2) as zv_pool,
                tc.tile_pool(name="ot", bufs=3) as ot_pool,
                tc.tile_pool(name="ps2", bufs=2, space="PSUM") as ps2,
            ):
                for h in range(HL):
                    for b in range(B):
                        for ci in range(SB):
                            tok_i0 = b * S + ci * CH
                            ps_o = ps2.tile([128, CH], F32, tag="po", bufs=2)
                            zv = zv_pool.tile([128, CH], F32, tag="zv")
                            njb = 4 * ci + 4
                            for jb in range(njb):
                                tok_j = slice(b * S + jb * 128, b * S + (jb + 1) * 128)
                                r = jb - 4 * ci  # diag position (>=0 on diagonal)
                                off = 128 * r if r > 0 else 0  # live query suffix
                                wid = CH - off
                                ps_s = ps2.tile([128, CH], F32, tag="ps", bufs=3)
                                nc.tensor.matmul(
                                    ps_s[:, :wid],
                                    kT_sb[:, h, tok_j],
                                    qT_sb[:, h, tok_i0 + off : tok_i0 + CH],
                                    start=True,
                                    stop=True,
                                )
                                pt = pt_pool.tile([128, CH], BF16, tag="pt")
                                nc.scalar.activation(pt[:, :wid], ps_s[:, :wid], EXP)
                                if r >= 0:
                                    # triangular corner: queries [128r, 128r+128)
                                    nc.vector.tensor_tensor(
                                        pt[:, :128], pt[:, :128], mask_sb[:], MUL
                                    )
                                if jb == 0:
                                    nc.vector.tensor_copy(zv[:], pt[:])
                                else:
                                    nc.vector.tensor_tensor(
                                        zv[:, off:], zv[:, off:], pt[:, :wid], ADD
                                    )
                                vb = b * (S // 128) + jb
                                nc.tensor.matmul(
                                    ps_o[:, off:],
                                    v_sb[:, vb, h * 128 : (h + 1) * 128],
                                    pt[:, :wid],
                                    start=(jb == 0),
                                    stop=(jb == njb - 1),
                                )
                            # normalize by 1/Z (partition-sum via ones matmul,
                            # partition-broadcast via K=1 matmul)
                            zvb = pt_pool.tile([128, CH], BF16, tag="zvb")
                            nc.vector.tensor_copy(zvb[:], zv[:])
                            ps_z = ps2.tile([1, CH], F32, tag="pz", bufs=1)
                            nc.tensor.matmul(
                                ps_z[:], ones_col[:], zvb[:], start=True, stop=True
                            )
                            rz = ot_pool.tile([1, CH], F32, tag="rz")
                            nc.vector.reciprocal_approx_fast(rz[:], ps_z[:])
                            rzb = ot_pool.tile([1, CH], BF16, tag="rzb")
                            nc.vector.tensor_copy(rzb[:], rz[:])
                            ps_bc = ps2.tile([128, CH], F32, tag="pbc", bufs=1)
                            nc.tensor.matmul(
                                ps_bc[:], ones_row[:], rzb[:], start=True, stop=True
                            )
                            bc_sb = ot_pool.tile([128, CH], F32, tag="bc_sb")
                            nc.vector.tensor_copy(bc_sb[:], ps_bc[:])
                            otn = ot_pool.tile([128, CH], BF16, tag="otn")
                            nc.vector.tensor_tensor(otn[:], ps_o[:], bc_sb[:], MUL)
                            sh = b * SB + ci
                            nc.sync.dma_start(a2a_in[h][sh, :, :], otn[:])

                    # ---- per-head AllToAll: head 0's collective overlaps head
                    # 1's attention compute; head 1's overlaps phase A below
                    nc.gpsimd.collective_compute(
                        "AllToAll",
                        mybir.AluOpType.bypass,
                        replica_groups=[list(range(W))],
                        ins=[a2a_in[h].opt()],
                        outs=[a2a_out[h].opt()],
                    )
                    # pull this head's row tiles into SBUF right away
                    src = a2a_out[h][:].rearrange("w d c -> (w d) c")
                    for blk in range(W):
                        nc.sync.dma_start(
                            attn_t[2 * blk + h][:], src[blk * 128 : (blk + 1) * 128, :]
                        )

            # ============ stage 4: output projection for this core's rows =====
            # Two phases so ALL even-k (head-0) matmuls can run while the
            # second AllToAll is still in flight: phase A accumulates even
            # k-tiles for every (n, m) output group and parks the partial sums
            # in SBUF; phase B adds the odd k-tiles and writes out.
            with (
                tc.tile_pool(name="par", bufs=16) as par_pool,
                tc.tile_pool(name="oev", bufs=3) as oev_pool,
                tc.tile_pool(name="ps4", bufs=4, space="PSUM") as ps4,
            ):
                NCHUNK = D // CH
                MS = RPC // 128
                partial = {}
                for n in range(NCHUNK):
                    for m in range(MS):
                        ps_out = ps4.tile([128, CH], F32, tag="pout", name="ps_outA")
                        for i, kt in enumerate(range(0, KT, 2)):
                            nc.tensor.matmul(
                                ps_out[:],
                                attn_t[kt][:, m * 128 : (m + 1) * 128],
                                wts[(n, kt)][:],
                                start=(i == 0),
                                stop=(i == KT // 2 - 1),
                            )
                        par = par_pool.tile(
                            [128, CH], F32, tag="par", name=f"par{n}_{m}"
                        )
                        nc.vector.tensor_copy(par[:], ps_out[:])
                        partial[(n, m)] = par
                for n in range(NCHUNK):
                    for m in range(MS):
                        ps_out = ps4.tile([128, CH], F32, tag="pout", name="ps_outB")
                        for i, kt in enumerate(range(1, KT, 2)):
                            nc.tensor.matmul(
                                ps_out[:],
                                attn_t[kt][:, m * 128 : (m + 1) * 128],
                                wts[(n, kt)][:],
                                start=(i == 0),
                                stop=(i == KT // 2 - 1),
                            )
                        oev = oev_pool.tile([128, CH], F32, tag="oev")
                        nc.vector.tensor_tensor(
                            oev[:], ps_out[:], partial[(n, m)][:], ADD
                        )
                        nc.sync.dma_start(
                            out[m * 128 : (m + 1) * 128, n * CH : (n + 1) * CH],
                            oev[:],
                        )

        if dbg is not None:
            nc.sync.dma_start(dbg["dbg_q"], qT_sb[:])
            nc.sync.dma_start(dbg["dbg_k"], kT_sb[:])
            nc.sync.dma_start(dbg["dbg_v"], v_sb[:])
            for kt in range(KT):
                nc.sync.dma_start(dbg["dbg_attn"][:, kt, :], attn_t[kt][:])


_NC_CACHE = None


def _get_nc():
    global _NC_CACHE
    if _NC_CACHE is None:
        _NC_CACHE = build_nc()
    return _NC_CACHE


def _prep_inputs(x, freq_cos, freq_sin, wq, wk, wv, wo):
    bf = ml_dtypes.bfloat16
    x = np.asarray(x, np.float32).reshape(N, D)
    fc = np.asarray(freq_cos, np.float32)  # [S, 64]
    fs = np.asarray(freq_sin, np.float32)
    wq = np.asarray(wq, np.float32)
    wk = np.asarray(wk, np.float32)
    wv = np.asarray(wv, np.float32)
    wo = np.asarray(wo, np.float32)

    xT = np.ascontiguousarray(x.T).astype(bf)  # [D, N]
    woT = np.ascontiguousarray(wo.T).astype(bf)  # [D, D]

    # RoPE tables, expanded to the full head dim and tiled over batch.
    # fc2[d, b*S+i] = cos(freq[i, d//2]); fss carries sin with the sign of the
    # pair-swap term: -sin for even d, +sin for odd d.
    fc2 = np.tile(np.repeat(fc.T, 2, axis=0), (1, B)).astype(np.float32)
    sgn = np.where(np.arange(HD) % 2 == 0, -1.0, 1.0).astype(np.float32)[:, None]
    fss = (np.tile(np.repeat(fs.T, 2, axis=0), (1, B)) * sgn).astype(np.float32)
    fc2 = np.ascontiguousarray(fc2)
    fss = np.ascontiguousarray(fss)

    pswap = np.zeros((HD, HD), np.float32)
    pswap[np.arange(HD) ^ 1, np.arange(HD)] = 1.0
    pswap = pswap.astype(bf)

    # triangular 0/1 mask for the 128x128 diagonal corner: allow j <= i
    jp = np.arange(128)[:, None]
    ii = np.arange(128)[None, :]
    mask01 = (jp <= ii).astype(np.float32).astype(bf)

    scale = 1.0 / np.sqrt(HD)
    in_maps = []
    for c in range(W):
        rows = slice(c * DL, (c + 1) * DL)
        wqT = np.ascontiguousarray((wq[rows] * scale).T).astype(bf)
        wkT = np.ascontiguousarray(wk[rows].T).astype(bf)
        wvT = np.ascontiguousarray(wv[rows].T).astype(bf)
        in_maps.append(
            {
                "xT": xT,
                "wqT": wqT,
                "wkT": wkT,
                "wvT": wvT,
                "woT": woT,
                "fc2": fc2,
                "fss": fss,
                "pswap": pswap,
                "mask01": mask01,
            }
        )
    return in_maps


def kernel(x, freq_cos, freq_sin, wq, wk, wv, wo, _trace=False, _trace_kwargs=None):
    nc = _get_nc()
    in_maps = _prep_inputs(x, freq_cos, freq_sin, wq, wk, wv, wo)
    kwargs = {}
    if _trace:
        kwargs.update(trace=True, **(_trace_kwargs or {}))
    res = run_bass_kernel_spmd(nc, in_maps, core_ids=list(range(W)), **kwargs)
    kernel.last_result = res
    full = np.concatenate([res.results[c]["out"] for c in range(W)], axis=0)
    return full.reshape(B, S, D).astype(np.float32)


# revision 19
# speedup vs baseline: 1.3801x; 1.1091x over previous
"""Distributed Trainium2 Bass kernel for causal multi-head attention (RoPE).

Reference computation (B=2, S=2048, D=2048, H=16, hd=128):
    q/k/v = x @ w{q,k,v}.T ; rope(q, k) ; causal softmax attention ; out @ wo.T

Sharding over 8 NeuronCores (tensor-parallel over heads, then rows):
  - Each core owns 2 heads: computes its q/k/v projections (256 features),
    RoPE, and causal attention for those heads.
  - Attention outputs (normalized by the softmax denominator via a broadcast
    trick) are exchanged with one AllToAll per local head so each core ends
    up with ALL features for 1/8 of the token rows; the per-head split lets
    the first collective overlap the second head's attention compute.
  - Each core computes its 512 rows of the output projection; the host
    concatenates the 8 row-chunks.

Everything is computed in bf16 on the TensorEngine with f32 PSUM
accumulation; softmax runs without max-subtraction (scores are O(1) by
construction) with the causal mask applied as a 0/1 multiply after exp.

Layout tricks:
  - Activations live feature-major (xT, qT, kT) so matmul contractions are
    natural; v is produced token-major directly by swapping matmul operands.
  - Scores are computed transposed (sT[j, i]) so no on-chip transposes of the
    softmax matrix are needed; softmax sums over partitions use a ones-vector
    matmul; the per-token 1/Z is broadcast across partitions with a K=1
    matmul.
  - RoPE pair-swap (partition crossing) is done with a permutation-matrix
    matmul; cos/sin tables are pre-expanded on the host.
  - Causal structure: fully-masked j-blocks are skipped; on the 4 diagonal
    j-blocks of each 512-wide i-chunk only the live suffix of queries is
    computed, so just one triangular 128x128 corner needs the 0/1 mask.
  - The output projection runs in two phases (even k-tiles, then odd) with
    partial sums parked in SBUF, so a full pass of matmuls is available to
    overlap the second AllToAll.
"""

import numpy as np
import ml_dtypes

import concourse.mybir as mybir
import concourse.tile as tile
from concourse import bacc
from concourse.bass_utils import run_bass_kernel_spmd

# Problem constants (hardcoded per harness contract)
B, S, D, H = 2, 2048, 2048, 16
W = 8  # cores
N = B * S  # 4096 tokens
HD = D // H  # 128 head dim
HL = H // W  # 2 heads per core
DL = HL * HD  # 256 features per core
CH = 512  # token chunk
NCH = N // CH  # 8 chunks
KT = D // 128  # 16 contraction tiles
RPC = N // W  # 512 rows per core for the output projection
NVB = N // 128  # 32 v token-blocks
SB = S // CH  # 4 i-chunks per batch

F32 = mybir.dt.float32
BF16 = mybir.dt.bfloat16
MUL = mybir.AluOpType.mult
ADD = mybir.AluOpType.add


def build_nc(dumps=False):
    nc = bacc.Bacc("TRN2", target_bir_lowering=False, debug=False, num_devices=W)

    xT = nc.dram_tensor("xT", [D, N], BF16, kind="ExternalInput").ap()
    wqT = nc.dram_tensor("wqT", [D, DL], BF16, kind="ExternalInput").ap()
    wkT = nc.dram_tensor("wkT", [D, DL], BF16, kind="ExternalInput").ap()
    wvT = nc.dram_tensor("wvT", [D, DL], BF16, kind="ExternalInput").ap()
    woT = nc.dram_tensor("woT", [D, D], BF16, kind="ExternalInput").ap()
    fc2 = nc.dram_tensor("fc2", [HD, N], F32, kind="ExternalInput").ap()
    fss = nc.dram_tensor("fss", [HD, N], F32, kind="ExternalInput").ap()
    pswap = nc.dram_tensor("pswap", [HD, HD], BF16, kind="ExternalInput").ap()
    mask01 = nc.dram_tensor("mask01", [128, 128], BF16, kind="ExternalInput").ap()
    out = nc.dram_tensor("out", [RPC, D], F32, kind="ExternalOutput").ap()

    dbg = None
    if dumps:
        dbg = {
            "dbg_q": nc.dram_tensor("dbg_q", [128, HL, N], BF16, kind="ExternalOutput").ap(),
            "dbg_k": nc.dram_tensor("dbg_k", [128, HL, N], BF16, kind="ExternalOutput").ap(),
            "dbg_v": nc.dram_tensor("dbg_v", [128, NVB, DL], BF16, kind="ExternalOutput").ap(),
            "dbg_attn": nc.dram_tensor("dbg_attn", [128, KT, CH], BF16, kind="ExternalOutput").ap(),
        }

    with tile.TileContext(nc) as tc:
        _body(tc, xT, wqT, wkT, wvT, woT, fc2, fss, pswap, mask01, out, dbg)

    nc.compile()
    return nc


def _body(tc, xT, wqT, wkT, wvT, woT, fc2, fss, pswap, mask01, out, dbg=None):
    nc = tc.nc
    EXP = mybir.ActivationFunctionType.Exp

    with (
        tc.tile_pool(name="const", bufs=1) as const,
        tc.tile_pool(name="dram", bufs=1, space="DRAM") as dram,
    ):
        # ---- persistent SBUF state (weights first: stage 1 needs them now) ----
        wq_sb = const.tile([128, KT, DL], BF16)
        wk_sb = const.tile([128, KT, DL], BF16)
        wv_sb = const.tile([128, KT, DL], BF16)
        nc.sync.dma_start(wq_sb[:], wqT.rearrange("(kt p) m -> p kt m", p=128))
        nc.sync.dma_start(wk_sb[:], wkT.rearrange("(kt p) m -> p kt m", p=128))
        nc.sync.dma_start(wv_sb[:], wvT.rearrange("(kt p) m -> p kt m", p=128))
        pswap_sb = const.tile([128, 128], BF16)
        nc.sync.dma_start(pswap_sb[:], pswap)
        mask_sb = const.tile([128, 128], BF16)
        ones_col = const.tile([128, 1], BF16)
        nc.vector.memset(ones_col[:], 1.0)
        ones_row = const.tile([1, 128], BF16)
        nc.vector.memset(ones_row[:], 1.0)

        qT_sb = const.tile([128, HL, N], BF16)  # feature-major q (post-rope)
        kT_sb = const.tile([128, HL, N], BF16)
        v_sb = const.tile([128, NVB, DL], BF16)  # token-major v
        # post-A2A row tiles, feature-major; one tile per k-tile so phase-A
        # matmuls only depend on the first AllToAll's DMAs
        attn_t = [
            const.tile([128, CH], BF16, name=f"attn_t{kt}") for kt in range(KT)
        ]

        # per-head A2A buffers (shard s of head h = oT for rows [512s, 512s+512))
        a2a_in = [dram.tile([W, HD, CH], BF16, name=f"a2a_in{h}") for h in range(HL)]
        a2a_out = [dram.tile([W, HD, CH], BF16, name=f"a2a_out{h}") for h in range(HL)]

        # ================= stage 1: q/k/v projections + RoPE =================
        # K-contiguous per output tensor: all q matmuls for a chunk, then all
        # k, then all v (x tiles stay cached in SBUF).  Each tensor's PSUM
        # eviction then overlaps the next tensor's matmul phase, so chunk
        # boundaries don't stall the TensorEngine.
        with (
            tc.tile_pool(name="xin", bufs=20) as xin_pool,
            tc.tile_pool(name="ev", bufs=4) as ev_pool,
            tc.tile_pool(name="frq", bufs=1) as frq_pool,
            tc.tile_pool(name="ps1", bufs=1, space="PSUM") as ps1,
        ):
            fc2_sb = frq_pool.tile([128, N], F32)
            fss_sb = frq_pool.tile([128, N], F32)

            def rope_evict(ps_t, sub, dst, tok):
                tmp = ev_pool.tile([128, CH], BF16, tag="tmp")
                nc.scalar.copy(tmp[:], ps_t[:])
                ps_sw = ps1.tile([128, CH], F32, tag=f"psw{sub}", name="ps_sw")
                nc.tensor.matmul(ps_sw[:], pswap_sb[:], tmp[:], start=True, stop=True)
                t1 = ev_pool.tile([128, CH], F32, tag="t1")
                t2 = ev_pool.tile([128, CH], F32, tag="t2")
                nc.vector.tensor_tensor(t1[:], ps_t[:], fc2_sb[:, tok], MUL)
                nc.vector.tensor_tensor(t2[:], ps_sw[:], fss_sb[:, tok], MUL)
                nc.vector.tensor_tensor(dst[:, sub, tok], t1[:], t2[:], ADD)

            for ch in range(NCH):
                tok = slice(ch * CH, (ch + 1) * CH)
                xts = []
                for kt in range(KT):
                    xt = xin_pool.tile([128, CH], BF16, tag="xt", name=f"xt{kt}")
                    nc.sync.dma_start(xt[:], xT[kt * 128 : (kt + 1) * 128, tok])
                    xts.append(xt)
                if ch == 0:
                    # issued after chunk-0 loads so they don't delay the first
                    # matmuls; only needed by the rope eviction below
                    nc.sync.dma_start(fc2_sb[:], fc2)
                    nc.sync.dma_start(fss_sb[:], fss)
                    nc.sync.dma_start(mask_sb[:], mask01)
                ps_q = [
                    ps1.tile([128, CH], F32, tag=f"pq{s}", name=f"ps_q{s}")
                    for s in range(2)
                ]
                ps_k = [
                    ps1.tile([128, CH], F32, tag=f"pk{s}", name=f"ps_k{s}")
                    for s in range(2)
                ]
                ps_v = [
                    ps1.tile([128, 2, 256], F32, tag=f"pv{s}", name=f"ps_v{s}")
                    for s in range(2)
                ]
                for kt in range(KT):
                    st, sp = kt == 0, kt == KT - 1
                    for sub in range(2):
                        fsl = slice(sub * 128, (sub + 1) * 128)
                        nc.tensor.matmul(
                            ps_q[sub][:], wq_sb[:, kt, fsl], xts[kt][:],
                            start=st, stop=sp,
                        )
                for sub in range(2):
                    rope_evict(ps_q[sub], sub, qT_sb, tok)
                for kt in range(KT):
                    st, sp = kt == 0, kt == KT - 1
                    for sub in range(2):
                        fsl = slice(sub * 128, (sub + 1) * 128)
                        nc.tensor.matmul(
                            ps_k[sub][:], wk_sb[:, kt, fsl], xts[kt][:],
                            start=st, stop=sp,
                        )
                for sub in range(2):
                    rope_evict(ps_k[sub], sub, kT_sb, tok)
                for kt in range(KT):
                    st, sp = kt == 0, kt == KT - 1
                    for t in range(4):
                        # start=True zeroes the whole 2KB PSUM bank, so only
                        # the bank's first slice may set it (kt==0, even t)
                        nc.tensor.matmul(
                            ps_v[t // 2][:, t % 2, :],
                            xts[kt][:, t * 128 : (t + 1) * 128],
                            wv_sb[:, kt, :],
                            start=(st and t % 2 == 0),
                            stop=sp,
                        )
                # evict v (token-major)
                for half in range(2):
                    nc.scalar.copy(
                        v_sb[:, ch * 4 + half * 2 : ch * 4 + half * 2 + 2, :],
                        ps_v[half][:],
                    )

        # preload ALL wo tiles on the gpsimd (SWDGE) queue so they stream in
        # during attention without delaying the sync-queue A2A staging writes
        with tc.tile_pool(name="wo", bufs=64) as wo_pool:
            wts = {}
            for n in range(D // CH):
                for kt in range(KT):
                    wt = wo_pool.tile([128, CH], BF16, tag="wo", name=f"wt{n}_{kt}")
                    nc.gpsimd.dma_start(
                        wt[:], woT[kt * 128 : (kt + 1) * 128, n * CH : (n + 1) * CH]
                    )
                    wts[(n, kt)] = wt

            # ================= stage 2: causal attention (head-outer) =========
            with (
                tc.tile_pool(name="pt", bufs=4) as pt_pool,
                tc.tile_pool(name="zv", bufs=2) as zv_pool,
                tc.tile_pool(name="ot", bufs=3) as ot_pool,
                tc.tile_pool(name="ps2", bufs=2, space="PSUM") as ps2,
            ):
                for h in range(HL):
                    for b in range(B):
                        for ci in range(SB):
                            tok_i0 = b * S + ci * CH
                            ps_o = ps2.tile([128, CH], F32, tag="po", bufs=2)
                            zv = zv_pool.tile([128, CH], F32, tag="zv")
                            njb = 4 * ci + 4
                            pend = []  # software pipeline: pv runs 1 jb behind s

                            def emit_pv(jb, pt, off, wid):
                                vb = b * (S // 128) + jb
                                nc.tensor.matmul(
                                    ps_o[:, off:],
                                    v_sb[:, vb, h * 128 : (h + 1) * 128],
                                    pt[:, :wid],
                                    start=(jb == 0),
                                    stop=(jb == njb - 1),
                                )

                            for jb in range(njb):
                                tok_j = slice(b * S + jb * 128, b * S + (jb + 1) * 128)
                                r = jb - 4 * ci  # diag position (>=0 on diagonal)
                                off = 128 * r if r > 0 else 0  # live query suffix
                                wid = CH - off
                                ps_s = ps2.tile([128, CH], F32, tag="ps", bufs=3)
                                nc.tensor.matmul(
                                    ps_s[:, :wid],
                                    kT_sb[:, h, tok_j],
                                    qT_sb[:, h, tok_i0 + off : tok_i0 + CH],
                                    start=True,
                                    stop=True,
                                )
                                pt = pt_pool.tile([128, CH], BF16, tag="pt")
                                nc.scalar.activation(pt[:, :wid], ps_s[:, :wid], EXP)
                                if r >= 0:
                                    # triangular corner: queries [128r, 128r+128)
                                    nc.vector.tensor_tensor(
                                        pt[:, :128], pt[:, :128], mask_sb[:], MUL
                                    )
                                if jb == 0:
                                    nc.vector.tensor_copy(zv[:], pt[:])
                                else:
                                    nc.vector.tensor_tensor(
                                        zv[:, off:], zv[:, off:], pt[:, :wid], ADD
                                    )
                                pend.append((jb, pt, off, wid))
                                if len(pend) > 1:
                                    emit_pv(*pend.pop(0))
                            while pend:
                                emit_pv(*pend.pop(0))
                            # normalize by 1/Z (partition-sum via ones matmul,
                            # partition-broadcast via K=1 matmul)
                            zvb = pt_pool.tile([128, CH], BF16, tag="zvb")
                            nc.vector.tensor_copy(zvb[:], zv[:])
                            ps_z = ps2.tile([1, CH], F32, tag="pz", bufs=1)
                            nc.tensor.matmul(
                                ps_z[:], ones_col[:], zvb[:], start=True, stop=True
                            )
                            rz = ot_pool.tile([1, CH], F32, tag="rz")
                            nc.vector.reciprocal_approx_fast(rz[:], ps_z[:])
                            rzb = ot_pool.tile([1, CH], BF16, tag="rzb")
                            nc.vector.tensor_copy(rzb[:], rz[:])
                            ps_bc = ps2.tile([128, CH], F32, tag="pbc", bufs=1)
                            nc.tensor.matmul(
                                ps_bc[:], ones_row[:], rzb[:], start=True, stop=True
                            )
                            bc_sb = ot_pool.tile([128, CH], F32, tag="bc_sb")
                            nc.vector.tensor_copy(bc_sb[:], ps_bc[:])
                            otn = ot_pool.tile([128, CH], BF16, tag="otn")
                            nc.vector.tensor_tensor(otn[:], ps_o[:], bc_sb[:], MUL)
                            sh = b * SB + ci
                            nc.sync.dma_start(a2a_in[h][sh, :, :], otn[:])

                    # ---- per-head AllToAll: head 0's collective overlaps head
                    # 1's attention compute; head 1's overlaps phase A below
                    nc.gpsimd.collective_compute(
                        "AllToAll",
                        mybir.AluOpType.bypass,
                        replica_groups=[list(range(W))],
                        ins=[a2a_in[h].opt()],
                        outs=[a2a_out[h].opt()],
                    )
                    # pull this head's row tiles into SBUF right away
                    src = a2a_out[h][:].rearrange("w d c -> (w d) c")
                    for blk in range(W):
                        nc.sync.dma_start(
                            attn_t[2 * blk + h][:], src[blk * 128 : (blk + 1) * 128, :]
                        )

            # ============ stage 4: output projection for this core's rows =====
            # Two phases so ALL even-k (head-0) matmuls can run while the
            # second AllToAll is still in flight: phase A accumulates even
            # k-tiles for every (n, m) output group and parks the partial sums
            # in SBUF; phase B adds the odd k-tiles and writes out.
            with (
                tc.tile_pool(name="par", bufs=16) as par_pool,
                tc.tile_pool(name="oev", bufs=3) as oev_pool,
                tc.tile_pool(name="ps4", bufs=4, space="PSUM") as ps4,
            ):
                NCHUNK = D // CH
                MS = RPC // 128
                partial = {}
                for n in range(NCHUNK):
                    for m in range(MS):
                        ps_out = ps4.tile([128, CH], F32, tag="pout", name="ps_outA")
                        for i, kt in enumerate(range(0, KT, 2)):
                            nc.tensor.matmul(
                                ps_out[:],
                                attn_t[kt][:, m * 128 : (m + 1) * 128],
                                wts[(n, kt)][:],
                                start=(i == 0),
                                stop=(i == KT // 2 - 1),
                            )
                        par = par_pool.tile(
                            [128, CH], F32, tag="par", name=f"par{n}_{m}"
                        )
                        nc.vector.tensor_copy(par[:], ps_out[:])
                        partial[(n, m)] = par
                for n in range(NCHUNK):
                    for m in range(MS):
                        ps_out = ps4.tile([128, CH], F32, tag="pout", name="ps_outB")
                        for i, kt in enumerate(range(1, KT, 2)):
                            nc.tensor.matmul(
                                ps_out[:],
                                attn_t[kt][:, m * 128 : (m + 1) * 128],
                                wts[(n, kt)][:],
                                start=(i == 0),
                                stop=(i == KT // 2 - 1),
                            )
                        oev = oev_pool.tile([128, CH], F32, tag="oev")
                        nc.vector.tensor_tensor(
                            oev[:], ps_out[:], partial[(n, m)][:], ADD
                        )
                        nc.sync.dma_start(
                            out[m * 128 : (m + 1) * 128, n * CH : (n + 1) * CH],
                            oev[:],
                        )

        if dbg is not None:
            nc.sync.dma_start(dbg["dbg_q"], qT_sb[:])
            nc.sync.dma_start(dbg["dbg_k"], kT_sb[:])
            nc.sync.dma_start(dbg["dbg_v"], v_sb[:])
            for kt in range(KT):
                nc.sync.dma_start(dbg["dbg_attn"][:, kt, :], attn_t[kt][:])


_NC_CACHE = None


def _get_nc():
    global _NC_CACHE
    if _NC_CACHE is None:
        _NC_CACHE = build_nc()
    return _NC_CACHE


def _prep_inputs(x, freq_cos, freq_sin, wq, wk, wv, wo):
    bf = ml_dtypes.bfloat16
    x = np.asarray(x, np.float32).reshape(N, D)
    fc = np.asarray(freq_cos, np.float32)  # [S, 64]
    fs = np.asarray(freq_sin, np.float32)
    wq = np.asarray(wq, np.float32)
    wk = np.asarray(wk, np.float32)
    wv = np.asarray(wv, np.float32)
    wo = np.asarray(wo, np.float32)

    xT = np.ascontiguousarray(x.T).astype(bf)  # [D, N]
    woT = np.ascontiguousarray(wo.T).astype(bf)  # [D, D]

    # RoPE tables, expanded to the full head dim and tiled over batch.
    # fc2[d, b*S+i] = cos(freq[i, d//2]); fss carries sin with the sign of the
    # pair-swap term: -sin for even d, +sin for odd d.
    fc2 = np.tile(np.repeat(fc.T, 2, axis=0), (1, B)).astype(np.float32)
    sgn = np.where(np.arange(HD) % 2 == 0, -1.0, 1.0).astype(np.float32)[:, None]
    fss = (np.tile(np.repeat(fs.T, 2, axis=0), (1, B)) * sgn).astype(np.float32)
    fc2 = np.ascontiguousarray(fc2)
    fss = np.ascontiguousarray(fss)

    pswap = np.zeros((HD, HD), np.float32)
    pswap[np.arange(HD) ^ 1, np.arange(HD)] = 1.0
    pswap = pswap.astype(bf)

    # triangular 0/1 mask for the 128x128 diagonal corner: allow j <= i
    jp = np.arange(128)[:, None]
    ii = np.arange(128)[None, :]
    mask01 = (jp <= ii).astype(np.float32).astype(bf)

    scale = 1.0 / np.sqrt(HD)
    in_maps = []
    for c in range(W):
        rows = slice(c * DL, (c + 1) * DL)
        wqT = np.ascontiguousarray((wq[rows] * scale).T).astype(bf)
        wkT = np.ascontiguousarray(wk[rows].T).astype(bf)
        wvT = np.ascontiguousarray(wv[rows].T).astype(bf)
        in_maps.append(
            {
                "xT": xT,
                "wqT": wqT,
                "wkT": wkT,
                "wvT": wvT,
                "woT": woT,
                "fc2": fc2,
                "fss": fss,
                "pswap": pswap,
                "mask01": mask01,
            }
        )
    return in_maps


def kernel(x, freq_cos, freq_sin, wq, wk, wv, wo, _trace=False, _trace_kwargs=None):
    nc = _get_nc()
    in_maps = _prep_inputs(x, freq_cos, freq_sin, wq, wk, wv, wo)
    kwargs = {}
    if _trace:
        kwargs.update(trace=True, **(_trace_kwargs or {}))
    res = run_bass_kernel_spmd(nc, in_maps, core_ids=list(range(W)), **kwargs)
    kernel.last_result = res
    full = np.concatenate([res.results[c]["out"] for c in range(W)], axis=0)
    return full.reshape(B, S, D).astype(np.float32)


# revision 23
# speedup vs baseline: 1.3834x; 1.0024x over previous
"""Distributed Trainium2 Bass kernel for causal multi-head attention (RoPE).

Reference computation (B=2, S=2048, D=2048, H=16, hd=128):
    q/k/v = x @ w{q,k,v}.T ; rope(q, k) ; causal softmax attention ; out @ wo.T

Sharding over 8 NeuronCores (tensor-parallel over heads, then rows):
  - Each core owns 2 heads: computes its q/k/v projections (256 features),
    RoPE, and causal attention for those heads.
  - Attention outputs (normalized by the softmax denominator via a broadcast
    trick) are exchanged with one AllToAll per local head so each core ends
    up with ALL features for 1/8 of the token rows; the per-head split lets
    the first collective overlap the second head's attention compute.
  - Each core computes its 512 rows of the output projection; the host
    concatenates the 8 row-chunks.

Everything is computed in bf16 on the TensorEngine with f32 PSUM
accumulation; softmax runs without max-subtraction (scores are O(1) by
construction) with the causal mask applied as a 0/1 multiply after exp.

Layout tricks:
  - Activations live feature-major (xT, qT, kT) so matmul contractions are
    natural; v is produced token-major directly by swapping matmul operands.
  - Scores are computed transposed (sT[j, i]) so no on-chip transposes of the
    softmax matrix are needed; softmax sums over partitions use a ones-vector
    matmul; the per-token 1/Z is broadcast across partitions with a K=1
    matmul.
  - RoPE pair-swap (partition crossing) is done with a permutation-matrix
    matmul; cos/sin tables are pre-expanded on the host.
  - Causal structure: fully-masked j-blocks are skipped; on the 4 diagonal
    j-blocks of each 512-wide i-chunk only the live suffix of queries is
    computed, so just one triangular 128x128 corner needs the 0/1 mask.
  - The output projection runs in two phases (even k-tiles, then odd) with
    partial sums parked in SBUF, so a full pass of matmuls is available to
    overlap the second AllToAll.
"""

import numpy as np
import ml_dtypes

import concourse.mybir as mybir
import concourse.tile as tile
from concourse import bacc
from concourse.bass_utils import run_bass_kernel_spmd

# Problem constants (hardcoded per harness contract)
B, S, D, H = 2, 2048, 2048, 16
W = 8  # cores
N = B * S  # 4096 tokens
HD = D // H  # 128 head dim
HL = H // W  # 2 heads per core
DL = HL * HD  # 256 features per core
CH = 512  # token chunk
NCH = N // CH  # 8 chunks
KT = D // 128  # 16 contraction tiles
RPC = N // W  # 512 rows per core for the output projection
NVB = N // 128  # 32 v token-blocks
SB = S // CH  # 4 i-chunks per batch

F32 = mybir.dt.float32
BF16 = mybir.dt.bfloat16
MUL = mybir.AluOpType.mult
ADD = mybir.AluOpType.add


def build_nc(dumps=False):
    nc = bacc.Bacc("TRN2", target_bir_lowering=False, debug=False, num_devices=W)

    xT = nc.dram_tensor("xT", [D, N], BF16, kind="ExternalInput").ap()
    wqT = nc.dram_tensor("wqT", [D, DL], BF16, kind="ExternalInput").ap()
    wkT = nc.dram_tensor("wkT", [D, DL], BF16, kind="ExternalInput").ap()
    wvT = nc.dram_tensor("wvT", [D, DL], BF16, kind="ExternalInput").ap()
    woT = nc.dram_tensor("woT", [D, D], BF16, kind="ExternalInput").ap()
    fc2 = nc.dram_tensor("fc2", [HD, N], F32, kind="ExternalInput").ap()
    fss = nc.dram_tensor("fss", [HD, N], F32, kind="ExternalInput").ap()
    pswap = nc.dram_tensor("pswap", [HD, HD], BF16, kind="ExternalInput").ap()
    mask01 = nc.dram_tensor("mask01", [128, 128], BF16, kind="ExternalInput").ap()
    out = nc.dram_tensor("out", [RPC, D], F32, kind="ExternalOutput").ap()

    dbg = None
    if dumps:
        dbg = {
            "dbg_q": nc.dram_tensor("dbg_q", [128, HL, N], BF16, kind="ExternalOutput").ap(),
            "dbg_k": nc.dram_tensor("dbg_k", [128, HL, N], BF16, kind="ExternalOutput").ap(),
            "dbg_v": nc.dram_tensor("dbg_v", [128, NVB, DL], BF16, kind="ExternalOutput").ap(),
            "dbg_attn": nc.dram_tensor("dbg_attn", [128, KT, CH], BF16, kind="ExternalOutput").ap(),
        }

    with tile.TileContext(nc) as tc:
        _body(tc, xT, wqT, wkT, wvT, woT, fc2, fss, pswap, mask01, out, dbg)

    nc.compile()
    return nc


def _body(tc, xT, wqT, wkT, wvT, woT, fc2, fss, pswap, mask01, out, dbg=None):
    nc = tc.nc
    EXP = mybir.ActivationFunctionType.Exp

    with (
        tc.tile_pool(name="const", bufs=1) as const,
        tc.tile_pool(name="dram", bufs=1, space="DRAM") as dram,
    ):
        # ---- persistent SBUF state (weights first: stage 1 needs them now) ----
        # weights as half-tiles (kt 0-7 / 8-15) so the first matmuls only wait
        # on the first half-load
        wq_sb, wk_sb, wv_sb = {}, {}, {}
        for half in range(2):
            ksl = slice(half * 8 * 128, (half + 1) * 8 * 128)
            for d, src, nm in ((wq_sb, wqT, "q"), (wk_sb, wkT, "k"), (wv_sb, wvT, "v")):
                t = const.tile([128, 8, DL], BF16, name=f"w{nm}_h{half}")
                nc.sync.dma_start(t[:], src[ksl, :].rearrange("(kt p) m -> p kt m", p=128))
                d[half] = t
        pswap_sb = const.tile([128, 128], BF16)
        nc.sync.dma_start(pswap_sb[:], pswap)
        mask_sb = const.tile([128, 128], BF16)
        ones_col = const.tile([128, 1], BF16)
        nc.vector.memset(ones_col[:], 1.0)
        ones_row = const.tile([1, 128], BF16)
        nc.vector.memset(ones_row[:], 1.0)

        qT_sb = const.tile([128, HL, N], BF16)  # feature-major q (post-rope)
        kT_sb = const.tile([128, HL, N], BF16)
        v_sb = const.tile([128, NVB, DL], BF16)  # token-major v
        # post-A2A row tiles, feature-major; one tile per k-tile so phase-A
        # matmuls only depend on the first AllToAll's DMAs
        attn_t = [
            const.tile([128, CH], BF16, name=f"attn_t{kt}") for kt in range(KT)
        ]

        # per-head A2A buffers (shard s of head h = oT for rows [512s, 512s+512))
        a2a_in = [dram.tile([W, HD, CH], BF16, name=f"a2a_in{h}") for h in range(HL)]
        a2a_out = [dram.tile([W, HD, CH], BF16, name=f"a2a_out{h}") for h in range(HL)]

        # ================= stage 1: q/k/v projections + RoPE =================
        # K-contiguous per output tensor: all q matmuls for a chunk, then all
        # k, then all v (x tiles stay cached in SBUF).  Each tensor's PSUM
        # eviction then overlaps the next tensor's matmul phase, so chunk
        # boundaries don't stall the TensorEngine.
        with (
            tc.tile_pool(name="xin", bufs=20) as xin_pool,
            tc.tile_pool(name="ev", bufs=4) as ev_pool,
            tc.tile_pool(name="frq", bufs=1) as frq_pool,
            tc.tile_pool(name="ps1", bufs=1, space="PSUM") as ps1,
        ):
            fc2_sb = frq_pool.tile([128, N], F32)
            fss_sb = frq_pool.tile([128, N], F32)

            def rope_evict(ps_t, sub, dst, tok):
                tmp = ev_pool.tile([128, CH], BF16, tag="tmp")
                nc.scalar.copy(tmp[:], ps_t[:])
                ps_sw = ps1.tile([128, CH], F32, tag=f"psw{sub}", name="ps_sw")
                nc.tensor.matmul(ps_sw[:], pswap_sb[:], tmp[:], start=True, stop=True)
                t1 = ev_pool.tile([128, CH], F32, tag="t1")
                t2 = ev_pool.tile([128, CH], F32, tag="t2")
                nc.vector.tensor_tensor(t1[:], ps_t[:], fc2_sb[:, tok], MUL)
                nc.vector.tensor_tensor(t2[:], ps_sw[:], fss_sb[:, tok], MUL)
                nc.vector.tensor_tensor(dst[:, sub, tok], t1[:], t2[:], ADD)

            for ch in range(NCH):
                tok = slice(ch * CH, (ch + 1) * CH)
                xts = []
                for kt in range(KT):
                    xt = xin_pool.tile([128, CH], BF16, tag="xt", name=f"xt{kt}")
                    nc.sync.dma_start(xt[:], xT[kt * 128 : (kt + 1) * 128, tok])
                    xts.append(xt)
                if ch == 0:
                    # issued after chunk-0 loads so they don't delay the first
                    # matmuls; only needed by the rope eviction below
                    nc.sync.dma_start(fc2_sb[:], fc2)
                    nc.sync.dma_start(fss_sb[:], fss)
                    nc.sync.dma_start(mask_sb[:], mask01)
                ps_q = [
                    ps1.tile([128, CH], F32, tag=f"pq{s}", name=f"ps_q{s}")
                    for s in range(2)
                ]
                ps_k = [
                    ps1.tile([128, CH], F32, tag=f"pk{s}", name=f"ps_k{s}")
                    for s in range(2)
                ]
                ps_v = [
                    ps1.tile([128, 2, 256], F32, tag=f"pv{s}", name=f"ps_v{s}")
                    for s in range(2)
                ]
                for kt in range(KT):
                    st, sp = kt == 0, kt == KT - 1
                    for sub in range(2):
                        fsl = slice(sub * 128, (sub + 1) * 128)
                        nc.tensor.matmul(
                            ps_q[sub][:], wq_sb[kt // 8][:, kt % 8, fsl], xts[kt][:],
                            start=st, stop=sp,
                        )
                for sub in range(2):
                    rope_evict(ps_q[sub], sub, qT_sb, tok)
                for kt in range(KT):
                    st, sp = kt == 0, kt == KT - 1
                    for sub in range(2):
                        fsl = slice(sub * 128, (sub + 1) * 128)
                        nc.tensor.matmul(
                            ps_k[sub][:], wk_sb[kt // 8][:, kt % 8, fsl], xts[kt][:],
                            start=st, stop=sp,
                        )
                for sub in range(2):
                    rope_evict(ps_k[sub], sub, kT_sb, tok)
                for kt in range(KT):
                    st, sp = kt == 0, kt == KT - 1
                    for t in range(4):
                        # start=True zeroes the whole 2KB PSUM bank, so only
                        # the bank's first slice may set it (kt==0, even t)
                        nc.tensor.matmul(
                            ps_v[t // 2][:, t % 2, :],
                            xts[kt][:, t * 128 : (t + 1) * 128],
                            wv_sb[kt // 8][:, kt % 8, :],
                            start=(st and t % 2 == 0),
                            stop=sp,
                        )
                # evict v (token-major)
                for half in range(2):
                    nc.scalar.copy(
                        v_sb[:, ch * 4 + half * 2 : ch * 4 + half * 2 + 2, :],
                        ps_v[half][:],
                    )

        # preload ALL wo tiles on the gpsimd (SWDGE) queue so they stream in
        # during attention without delaying the sync-queue A2A staging writes
        with tc.tile_pool(name="wo", bufs=64) as wo_pool:
            wts = {}
            for n in range(D // CH):
                for kt in range(KT):
                    wt = wo_pool.tile([128, CH], BF16, tag="wo", name=f"wt{n}_{kt}")
                    nc.gpsimd.dma_start(
                        wt[:], woT[kt * 128 : (kt + 1) * 128, n * CH : (n + 1) * CH]
                    )
                    wts[(n, kt)] = wt

            # ================= stage 2: causal attention (head-outer) =========
            with (
                tc.tile_pool(name="pt", bufs=4) as pt_pool,
                tc.tile_pool(name="zv", bufs=2) as zv_pool,
                tc.tile_pool(name="ot", bufs=3) as ot_pool,
                tc.tile_pool(name="ps2", bufs=2, space="PSUM") as ps2,
            ):
                def emit_norm(h, b, ci, ps_o, zv):
                    # normalize by 1/Z (partition-sum via ones matmul,
                    # partition-broadcast via K=1 matmul)
                    zvb = pt_pool.tile([128, CH], BF16, tag="zvb")
                    nc.vector.tensor_copy(zvb[:], zv[:])
                    ps_z = ps2.tile([1, CH], F32, tag="pz", bufs=1)
                    nc.tensor.matmul(
                        ps_z[:], ones_col[:], zvb[:], start=True, stop=True
                    )
                    rz = ot_pool.tile([1, CH], F32, tag="rz")
                    nc.vector.reciprocal_approx_fast(rz[:], ps_z[:])
                    rzb = ot_pool.tile([1, CH], BF16, tag="rzb")
                    nc.vector.tensor_copy(rzb[:], rz[:])
                    ps_bc = ps2.tile([128, CH], F32, tag="pbc", bufs=1)
                    nc.tensor.matmul(
                        ps_bc[:], ones_row[:], rzb[:], start=True, stop=True
                    )
                    bc_sb = ot_pool.tile([128, CH], F32, tag="bc_sb")
                    nc.vector.tensor_copy(bc_sb[:], ps_bc[:])
                    otn = ot_pool.tile([128, CH], BF16, tag="otn")
                    nc.vector.tensor_tensor(otn[:], ps_o[:], bc_sb[:], MUL)
                    sh = b * SB + ci
                    nc.sync.dma_start(a2a_in[h][sh, :, :], otn[:])

                for h in range(HL):
                    pending_norm = None  # defer each group's Z-chain so the
                    # next group's score matmuls keep the PE fed meanwhile
                    for b in range(B):
                        for ci in range(SB):
                            tok_i0 = b * S + ci * CH
                            ps_o = ps2.tile([128, CH], F32, tag="po", bufs=2)
                            zv = zv_pool.tile([128, CH], F32, tag="zv")
                            njb = 4 * ci + 4
                            pend = []  # software pipeline: pv runs 2 jb behind s

                            def emit_pv(jb, pt, off, wid, ps_o=ps_o, njb=njb, b=b, h=h):
                                vb = b * (S // 128) + jb
                                nc.tensor.matmul(
                                    ps_o[:, off:],
                                    v_sb[:, vb, h * 128 : (h + 1) * 128],
                                    pt[:, :wid],
                                    start=(jb == 0),
                                    stop=(jb == njb - 1),
                                )

                            for jb in range(njb):
                                tok_j = slice(b * S + jb * 128, b * S + (jb + 1) * 128)
                                r = jb - 4 * ci  # diag position (>=0 on diagonal)
                                off = 128 * r if r > 0 else 0  # live query suffix
                                wid = CH - off
                                ps_s = ps2.tile([128, CH], F32, tag="ps", bufs=3)
                                nc.tensor.matmul(
                                    ps_s[:, :wid],
                                    kT_sb[:, h, tok_j],
                                    qT_sb[:, h, tok_i0 + off : tok_i0 + CH],
                                    start=True,
                                    stop=True,
                                )
                                pt = pt_pool.tile([128, CH], BF16, tag="pt")
                                nc.scalar.activation(pt[:, :wid], ps_s[:, :wid], EXP)
                                if r >= 0:
                                    # triangular corner: queries [128r, 128r+128)
                                    nc.vector.tensor_tensor(
                                        pt[:, :128], pt[:, :128], mask_sb[:], MUL
                                    )
                                if jb == 0:
                                    nc.vector.tensor_copy(zv[:], pt[:])
                                else:
                                    nc.vector.tensor_tensor(
                                        zv[:, off:], zv[:, off:], pt[:, :wid], ADD
                                    )
                                pend.append((jb, pt, off, wid))
                                if pending_norm is not None and jb == 1:
                                    emit_norm(*pending_norm)
                                    pending_norm = None
                                if len(pend) > 2:
                                    emit_pv(*pend.pop(0))
                            if pending_norm is not None:
                                emit_norm(*pending_norm)
                                pending_norm = None
                            while pend:
                                emit_pv(*pend.pop(0))
                            pending_norm = (h, b, ci, ps_o, zv)
                    emit_norm(*pending_norm)
                    pending_norm = None

                    # ---- per-head AllToAll: head 0's collective overlaps head
                    # 1's attention compute; head 1's overlaps phase A below
                    nc.gpsimd.collective_compute(
                        "AllToAll",
                        mybir.AluOpType.bypass,
                        replica_groups=[list(range(W))],
                        ins=[a2a_in[h].opt()],
                        outs=[a2a_out[h].opt()],
                    )
                    # pull this head's row tiles into SBUF right away
                    src = a2a_out[h][:].rearrange("w d c -> (w d) c")
                    for blk in range(W):
                        nc.sync.dma_start(
                            attn_t[2 * blk + h][:], src[blk * 128 : (blk + 1) * 128, :]
                        )

            # ============ stage 4: output projection for this core's rows =====
            # Two phases so ALL even-k (head-0) matmuls can run while the
            # second AllToAll is still in flight: phase A accumulates even
            # k-tiles for every (n, m) output group and parks the partial sums
            # in SBUF; phase B adds the odd k-tiles and writes out.
            with (
                tc.tile_pool(name="par", bufs=16) as par_pool,
                tc.tile_pool(name="oev", bufs=3) as oev_pool,
                tc.tile_pool(name="ps4", bufs=4, space="PSUM") as ps4,
            ):
                NCHUNK = D // CH
                MS = RPC // 128
                partial = {}
                for n in range(NCHUNK):
                    for m in range(MS):
                        ps_out = ps4.tile([128, CH], F32, tag="pout", name="ps_outA")
                        for i, kt in enumerate(range(0, KT, 2)):
                            nc.tensor.matmul(
                                ps_out[:],
                                attn_t[kt][:, m * 128 : (m + 1) * 128],
                                wts[(n, kt)][:],
                                start=(i == 0),
                                stop=(i == KT // 2 - 1),
                            )
                        par = par_pool.tile(
                            [128, CH], F32, tag="par", name=f"par{n}_{m}"
                        )
                        nc.vector.tensor_copy(par[:], ps_out[:])
                        partial[(n, m)] = par
                for n in range(NCHUNK):
                    for m in range(MS):
                        ps_out = ps4.tile([128, CH], F32, tag="pout", name="ps_outB")
                        for i, kt in enumerate(range(1, KT, 2)):
                            nc.tensor.matmul(
                                ps_out[:],
                                attn_t[kt][:, m * 128 : (m + 1) * 128],
                                wts[(n, kt)][:],
                                start=(i == 0),
                                stop=(i == KT // 2 - 1),
                            )
                        oev = oev_pool.tile([128, CH], F32, tag="oev")
                        nc.vector.tensor_tensor(
                            oev[:], ps_out[:], partial[(n, m)][:], ADD
                        )
                        nc.sync.dma_start(
                            out[m * 128 : (m + 1) * 128, n * CH : (n + 1) * CH],
                            oev[:],
                        )

        if dbg is not None:
            nc.sync.dma_start(dbg["dbg_q"], qT_sb[:])
            nc.sync.dma_start(dbg["dbg_k"], kT_sb[:])
            nc.sync.dma_start(dbg["dbg_v"], v_sb[:])
            for kt in range(KT):
                nc.sync.dma_start(dbg["dbg_attn"][:, kt, :], attn_t[kt][:])


_NC_CACHE = None


def _get_nc():
    global _NC_CACHE
    if _NC_CACHE is None:
        _NC_CACHE = build_nc()
    return _NC_CACHE


def _prep_inputs(x, freq_cos, freq_sin, wq, wk, wv, wo):
    bf = ml_dtypes.bfloat16
    x = np.asarray(x, np.float32).reshape(N, D)
    fc = np.asarray(freq_cos, np.float32)  # [S, 64]
    fs = np.asarray(freq_sin, np.float32)
    wq = np.asarray(wq, np.float32)
    wk = np.asarray(wk, np.float32)
    wv = np.asarray(wv, np.float32)
    wo = np.asarray(wo, np.float32)

    xT = np.ascontiguousarray(x.T).astype(bf)  # [D, N]
    woT = np.ascontiguousarray(wo.T).astype(bf)  # [D, D]

    # RoPE tables, expanded to the full head dim and tiled over batch.
    # fc2[d, b*S+i] = cos(freq[i, d//2]); fss carries sin with the sign of the
    # pair-swap term: -sin for even d, +sin for odd d.
    fc2 = np.tile(np.repeat(fc.T, 2, axis=0), (1, B)).astype(np.float32)
    sgn = np.where(np.arange(HD) % 2 == 0, -1.0, 1.0).astype(np.float32)[:, None]
    fss = (np.tile(np.repeat(fs.T, 2, axis=0), (1, B)) * sgn).astype(np.float32)
    fc2 = np.ascontiguousarray(fc2)
    fss = np.ascontiguousarray(fss)

    pswap = np.zeros((HD, HD), np.float32)
    pswap[np.arange(HD) ^ 1, np.arange(HD)] = 1.0
    pswap = pswap.astype(bf)

    # triangular 0/1 mask for the 128x128 diagonal corner: allow j <= i
    jp = np.arange(128)[:, None]
    ii = np.arange(128)[None, :]
    mask01 = (jp <= ii).astype(np.float32).astype(bf)

    scale = 1.0 / np.sqrt(HD)
    in_maps = []
    for c in range(W):
        rows = slice(c * DL, (c + 1) * DL)
        wqT = np.ascontiguousarray((wq[rows] * scale).T).astype(bf)
        wkT = np.ascontiguousarray(wk[rows].T).astype(bf)
        wvT = np.ascontiguousarray(wv[rows].T).astype(bf)
        in_maps.append(
            {
                "xT": xT,
                "wqT": wqT,
                "wkT": wkT,
                "wvT": wvT,
                "woT": woT,
                "fc2": fc2,
                "fss": fss,
                "pswap": pswap,
                "mask01": mask01,
            }
        )
    return in_maps


def kernel(x, freq_cos, freq_sin, wq, wk, wv, wo, _trace=False, _trace_kwargs=None):
    nc = _get_nc()
    in_maps = _prep_inputs(x, freq_cos, freq_sin, wq, wk, wv, wo)
    kwargs = {}
    if _trace:
        kwargs.update(trace=True, **(_trace_kwargs or {}))
    res = run_bass_kernel_spmd(nc, in_maps, core_ids=list(range(W)), **kwargs)
    kernel.last_result = res
    full = np.concatenate([res.results[c]["out"] for c in range(W)], axis=0)
    return full.reshape(B, S, D).astype(np.float32)


# revision 24
# speedup vs baseline: 1.4056x; 1.0161x over previous
"""Distributed Trainium2 Bass kernel for causal multi-head attention (RoPE).

Reference computation (B=2, S=2048, D=2048, H=16, hd=128):
    q/k/v = x @ w{q,k,v}.T ; rope(q, k) ; causal softmax attention ; out @ wo.T

Sharding over 8 NeuronCores (tensor-parallel over heads, then rows):
  - Each core owns 2 heads: computes its q/k/v projections (256 features),
    RoPE, and causal attention for those heads.
  - Attention outputs (normalized by the softmax denominator via a broadcast
    trick) are exchanged with one AllToAll per local head so each core ends
    up with ALL features for 1/8 of the token rows; the per-head split lets
    the first collective overlap the second head's attention compute.
  - Each core computes its 512 rows of the output projection; the host
    concatenates the 8 row-chunks.

Everything is computed in bf16 on the TensorEngine with f32 PSUM
accumulation; softmax runs without max-subtraction (scores are O(1) by
construction) with the causal mask applied as a 0/1 multiply after exp.

Layout tricks:
  - Activations live feature-major (xT, qT, kT) so matmul contractions are
    natural; v is produced token-major directly by swapping matmul operands.
  - Scores are computed transposed (sT[j, i]) so no on-chip transposes of the
    softmax matrix are needed; softmax sums over partitions use a ones-vector
    matmul; the per-token 1/Z is broadcast across partitions with a K=1
    matmul.
  - RoPE pair-swap (partition crossing) is done with a permutation-matrix
    matmul; cos/sin tables are pre-expanded on the host.
  - Causal structure: fully-masked j-blocks are skipped; on the 4 diagonal
    j-blocks of each 512-wide i-chunk only the live suffix of queries is
    computed, so just one triangular 128x128 corner needs the 0/1 mask.
  - The output projection runs in two phases (even k-tiles, then odd) with
    partial sums parked in SBUF, so a full pass of matmuls is available to
    overlap the second AllToAll.
"""

import numpy as np
import ml_dtypes

import concourse.mybir as mybir
import concourse.tile as tile
from concourse import bacc
from concourse.bass_utils import run_bass_kernel_spmd

# Problem constants (hardcoded per harness contract)
B, S, D, H = 2, 2048, 2048, 16
W = 8  # cores
N = B * S  # 4096 tokens
HD = D // H  # 128 head dim
HL = H // W  # 2 heads per core
DL = HL * HD  # 256 features per core
CH = 512  # token chunk
NCH = N // CH  # 8 chunks
KT = D // 128  # 16 contraction tiles
RPC = N // W  # 512 rows per core for the output projection
NVB = N // 128  # 32 v token-blocks
SB = S // CH  # 4 i-chunks per batch

F32 = mybir.dt.float32
BF16 = mybir.dt.bfloat16
MUL = mybir.AluOpType.mult
ADD = mybir.AluOpType.add


def build_nc(dumps=False):
    nc = bacc.Bacc("TRN2", target_bir_lowering=False, debug=False, num_devices=W)

    xT = nc.dram_tensor("xT", [D, N], BF16, kind="ExternalInput").ap()
    wqT = nc.dram_tensor("wqT", [D, DL], BF16, kind="ExternalInput").ap()
    wkT = nc.dram_tensor("wkT", [D, DL], BF16, kind="ExternalInput").ap()
    wvT = nc.dram_tensor("wvT", [D, DL], BF16, kind="ExternalInput").ap()
    woT = nc.dram_tensor("woT", [D, D], BF16, kind="ExternalInput").ap()
    fc2 = nc.dram_tensor("fc2", [HD, N], F32, kind="ExternalInput").ap()
    fss = nc.dram_tensor("fss", [HD, N], F32, kind="ExternalInput").ap()
    pswap = nc.dram_tensor("pswap", [HD, HD], BF16, kind="ExternalInput").ap()
    mask01 = nc.dram_tensor("mask01", [128, 128], BF16, kind="ExternalInput").ap()
    out = nc.dram_tensor("out", [RPC, D], F32, kind="ExternalOutput").ap()

    dbg = None
    if dumps:
        dbg = {
            "dbg_q": nc.dram_tensor("dbg_q", [128, HL, N], BF16, kind="ExternalOutput").ap(),
            "dbg_k": nc.dram_tensor("dbg_k", [128, HL, N], BF16, kind="ExternalOutput").ap(),
            "dbg_v": nc.dram_tensor("dbg_v", [128, NVB, DL], BF16, kind="ExternalOutput").ap(),
            "dbg_attn": nc.dram_tensor("dbg_attn", [128, KT, CH], BF16, kind="ExternalOutput").ap(),
        }

    with tile.TileContext(nc) as tc:
        _body(tc, xT, wqT, wkT, wvT, woT, fc2, fss, pswap, mask01, out, dbg)

    nc.compile()
    return nc


def _body(tc, xT, wqT, wkT, wvT, woT, fc2, fss, pswap, mask01, out, dbg=None):
    nc = tc.nc
    EXP = mybir.ActivationFunctionType.Exp

    with (
        tc.tile_pool(name="const", bufs=1) as const,
        tc.tile_pool(name="dram", bufs=1, space="DRAM") as dram,
    ):
        # ---- persistent SBUF state (weights first: stage 1 needs them now) ----
        # weights as half-tiles (kt 0-7 / 8-15) so the first matmuls only wait
        # on the first half-load
        wq_sb, wk_sb, wv_sb = {}, {}, {}
        for half in range(2):
            ksl = slice(half * 8 * 128, (half + 1) * 8 * 128)
            for d, src, nm in ((wq_sb, wqT, "q"), (wk_sb, wkT, "k"), (wv_sb, wvT, "v")):
                t = const.tile([128, 8, DL], BF16, name=f"w{nm}_h{half}")
                nc.sync.dma_start(t[:], src[ksl, :].rearrange("(kt p) m -> p kt m", p=128))
                d[half] = t
        pswap_sb = const.tile([128, 128], BF16)
        nc.sync.dma_start(pswap_sb[:], pswap)
        mask_sb = const.tile([128, 128], BF16)
        ones_col = const.tile([128, 1], BF16)
        nc.vector.memset(ones_col[:], 1.0)
        ones_row = const.tile([1, 128], BF16)
        nc.vector.memset(ones_row[:], 1.0)

        qT_sb = const.tile([128, HL, N], BF16)  # feature-major q (post-rope)
        kT_sb = const.tile([128, HL, N], BF16)
        v_sb = const.tile([128, NVB, DL], BF16)  # token-major v
        # post-A2A row tiles, feature-major; one tile per k-tile so phase-A
        # matmuls only depend on the first AllToAll's DMAs
        attn_t = [
            const.tile([128, CH], BF16, name=f"attn_t{kt}") for kt in range(KT)
        ]

        # per-head A2A buffers (shard s of head h = oT for rows [512s, 512s+512))
        a2a_in = [dram.tile([W, HD, CH], BF16, name=f"a2a_in{h}") for h in range(HL)]
        a2a_out = [dram.tile([W, HD, CH], BF16, name=f"a2a_out{h}") for h in range(HL)]

        # ================= stage 1: q/k/v projections + RoPE =================
        # K-contiguous per output tensor: all q matmuls for a chunk, then all
        # k, then all v (x tiles stay cached in SBUF).  Each tensor's PSUM
        # eviction then overlaps the next tensor's matmul phase, so chunk
        # boundaries don't stall the TensorEngine.
        with (
            tc.tile_pool(name="xin", bufs=20) as xin_pool,
            tc.tile_pool(name="ev", bufs=4) as ev_pool,
            tc.tile_pool(name="frq", bufs=1) as frq_pool,
            tc.tile_pool(name="ps1", bufs=1, space="PSUM") as ps1,
        ):
            fc2_sb = frq_pool.tile([128, N], F32)
            fss_sb = frq_pool.tile([128, N], F32)

            def rope_evict(ps_t, sub, dst, tok):
                tmp = ev_pool.tile([128, CH], BF16, tag="tmp")
                nc.scalar.copy(tmp[:], ps_t[:])
                ps_sw = ps1.tile([128, CH], F32, tag=f"psw{sub}", name="ps_sw")
                nc.tensor.matmul(ps_sw[:], pswap_sb[:], tmp[:], start=True, stop=True)
                t1 = ev_pool.tile([128, CH], F32, tag="t1")
                t2 = ev_pool.tile([128, CH], F32, tag="t2")
                nc.vector.tensor_tensor(t1[:], ps_t[:], fc2_sb[:, tok], MUL)
                nc.vector.tensor_tensor(t2[:], ps_sw[:], fss_sb[:, tok], MUL)
                nc.vector.tensor_tensor(dst[:, sub, tok], t1[:], t2[:], ADD)

            for ch in range(NCH):
                tok = slice(ch * CH, (ch + 1) * CH)
                xts = []
                for kt in range(KT):
                    xt = xin_pool.tile([128, CH], BF16, tag="xt", name=f"xt{kt}")
                    nc.sync.dma_start(xt[:], xT[kt * 128 : (kt + 1) * 128, tok])
                    xts.append(xt)
                if ch == 0:
                    # issued after chunk-0 loads so they don't delay the first
                    # matmuls; only needed by the rope eviction below
                    nc.sync.dma_start(fc2_sb[:], fc2)
                    nc.sync.dma_start(fss_sb[:], fss)
                    nc.sync.dma_start(mask_sb[:], mask01)
                ps_q = [
                    ps1.tile([128, CH], F32, tag=f"pq{s}", name=f"ps_q{s}")
                    for s in range(2)
                ]
                ps_k = [
                    ps1.tile([128, CH], F32, tag=f"pk{s}", name=f"ps_k{s}")
                    for s in range(2)
                ]
                ps_v = [
                    ps1.tile([128, 2, 256], F32, tag=f"pv{s}", name=f"ps_v{s}")
                    for s in range(2)
                ]
                for kt in range(KT):
                    st, sp = kt == 0, kt == KT - 1
                    for sub in range(2):
                        fsl = slice(sub * 128, (sub + 1) * 128)
                        nc.tensor.matmul(
                            ps_q[sub][:], wq_sb[kt // 8][:, kt % 8, fsl], xts[kt][:],
                            start=st, stop=sp,
                        )
                for sub in range(2):
                    rope_evict(ps_q[sub], sub, qT_sb, tok)
                for kt in range(KT):
                    st, sp = kt == 0, kt == KT - 1
                    for sub in range(2):
                        fsl = slice(sub * 128, (sub + 1) * 128)
                        nc.tensor.matmul(
                            ps_k[sub][:], wk_sb[kt // 8][:, kt % 8, fsl], xts[kt][:],
                            start=st, stop=sp,
                        )
                for sub in range(2):
                    rope_evict(ps_k[sub], sub, kT_sb, tok)
                for kt in range(KT):
                    st, sp = kt == 0, kt == KT - 1
                    for t in range(4):
                        # start=True zeroes the whole 2KB PSUM bank, so only
                        # the bank's first slice may set it (kt==0, even t)
                        nc.tensor.matmul(
                            ps_v[t // 2][:, t % 2, :],
                            xts[kt][:, t * 128 : (t + 1) * 128],
                            wv_sb[kt // 8][:, kt % 8, :],
                            start=(st and t % 2 == 0),
                            stop=sp,
                        )
                # evict v (token-major)
                for half in range(2):
                    nc.scalar.copy(
                        v_sb[:, ch * 4 + half * 2 : ch * 4 + half * 2 + 2, :],
                        ps_v[half][:],
                    )

        # preload ALL wo tiles on the gpsimd (SWDGE) queue so they stream in
        # during attention without delaying the sync-queue A2A staging writes
        with tc.tile_pool(name="wo", bufs=64) as wo_pool:
            wts = {}
            for n in range(D // CH):
                for kt in range(KT):
                    wt = wo_pool.tile([128, CH], BF16, tag="wo", name=f"wt{n}_{kt}")
                    nc.gpsimd.dma_start(
                        wt[:], woT[kt * 128 : (kt + 1) * 128, n * CH : (n + 1) * CH]
                    )
                    wts[(n, kt)] = wt

            # ================= stage 2: causal attention (head-outer) =========
            with (
                tc.tile_pool(name="pt", bufs=4) as pt_pool,
                tc.tile_pool(name="zv", bufs=2) as zv_pool,
                tc.tile_pool(name="ot", bufs=3) as ot_pool,
                tc.tile_pool(name="ps2", bufs=2, space="PSUM") as ps2,
            ):
                def emit_norm(h, b, ci, ps_o, zv):
                    # normalize by 1/Z (partition-sum via ones matmul,
                    # partition-broadcast via K=1 matmul)
                    zvb = pt_pool.tile([128, CH], BF16, tag="zvb")
                    nc.vector.tensor_copy(zvb[:], zv[:])
                    ps_z = ps2.tile([1, CH], F32, tag="pz", bufs=1)
                    nc.tensor.matmul(
                        ps_z[:], ones_col[:], zvb[:], start=True, stop=True
                    )
                    rz = ot_pool.tile([1, CH], F32, tag="rz")
                    nc.vector.reciprocal_approx_fast(rz[:], ps_z[:])
                    rzb = ot_pool.tile([1, CH], BF16, tag="rzb")
                    nc.vector.tensor_copy(rzb[:], rz[:])
                    ps_bc = ps2.tile([128, CH], F32, tag="pbc", bufs=1)
                    nc.tensor.matmul(
                        ps_bc[:], ones_row[:], rzb[:], start=True, stop=True
                    )
                    bc_sb = ot_pool.tile([128, CH], F32, tag="bc_sb")
                    nc.vector.tensor_copy(bc_sb[:], ps_bc[:])
                    otn = ot_pool.tile([128, CH], BF16, tag="otn")
                    nc.vector.tensor_tensor(otn[:], ps_o[:], bc_sb[:], MUL)
                    sh = b * SB + ci
                    nc.sync.dma_start(a2a_in[h][sh, :, :], otn[:])

                def emit_pv(jb, pt, off, wid, ps_o, njb, b, h):
                    vb = b * (S // 128) + jb
                    nc.tensor.matmul(
                        ps_o[:, off:],
                        v_sb[:, vb, h * 128 : (h + 1) * 128],
                        pt[:, :wid],
                        start=(jb == 0),
                        stop=(jb == njb - 1),
                    )

                for h in range(HL):
                    # software pipelines carried ACROSS group boundaries: pv
                    # matmuls run 2 j-blocks behind the score matmuls, and each
                    # group's Z-normalization chain is emitted a couple of
                    # score-steps into the next group, so the PE never waits on
                    # the exp/softmax chains
                    pend = []
                    pending_norm = None
                    norm_delay = 0
                    for b in range(B):
                        for ci in range(SB):
                            tok_i0 = b * S + ci * CH
                            ps_o = ps2.tile([128, CH], F32, tag="po", bufs=2)
                            zv = zv_pool.tile([128, CH], F32, tag="zv")
                            njb = 4 * ci + 4
                            for jb in range(njb):
                                tok_j = slice(b * S + jb * 128, b * S + (jb + 1) * 128)
                                r = jb - 4 * ci  # diag position (>=0 on diagonal)
                                off = 128 * r if r > 0 else 0  # live query suffix
                                wid = CH - off
                                ps_s = ps2.tile([128, CH], F32, tag="ps", bufs=3)
                                nc.tensor.matmul(
                                    ps_s[:, :wid],
                                    kT_sb[:, h, tok_j],
                                    qT_sb[:, h, tok_i0 + off : tok_i0 + CH],
                                    start=True,
                                    stop=True,
                                )
                                pt = pt_pool.tile([128, CH], BF16, tag="pt")
                                nc.scalar.activation(pt[:, :wid], ps_s[:, :wid], EXP)
                                if r >= 0:
                                    # triangular corner: queries [128r, 128r+128)
                                    nc.vector.tensor_tensor(
                                        pt[:, :128], pt[:, :128], mask_sb[:], MUL
                                    )
                                if jb == 0:
                                    nc.vector.tensor_copy(zv[:], pt[:])
                                else:
                                    nc.vector.tensor_tensor(
                                        zv[:, off:], zv[:, off:], pt[:, :wid], ADD
                                    )
                                pend.append((jb, pt, off, wid, ps_o, njb, b, h))
                                if len(pend) > 2:
                                    emit_pv(*pend.pop(0))
                                if pending_norm is not None:
                                    norm_delay -= 1
                                    if norm_delay <= 0:
                                        emit_norm(*pending_norm)
                                        pending_norm = None
                            pending_norm = (h, b, ci, ps_o, zv)
                            norm_delay = 4  # all of this group's pv matmuls
                            # have drained from `pend` after 2 more steps
                    while pend:
                        emit_pv(*pend.pop(0))
                    if pending_norm is not None:
                        emit_norm(*pending_norm)
                        pending_norm = None

                    # ---- per-head AllToAll: head 0's collective overlaps head
                    # 1's attention compute; head 1's overlaps phase A below
                    nc.gpsimd.collective_compute(
                        "AllToAll",
                        mybir.AluOpType.bypass,
                        replica_groups=[list(range(W))],
                        ins=[a2a_in[h].opt()],
                        outs=[a2a_out[h].opt()],
                    )
                    # pull this head's row tiles into SBUF right away
                    src = a2a_out[h][:].rearrange("w d c -> (w d) c")
                    for blk in range(W):
                        nc.sync.dma_start(
                            attn_t[2 * blk + h][:], src[blk * 128 : (blk + 1) * 128, :]
                        )

            # ============ stage 4: output projection for this core's rows =====
            # Two phases so ALL even-k (head-0) matmuls can run while the
            # second AllToAll is still in flight: phase A accumulates even
            # k-tiles for every (n, m) output group and parks the partial sums
            # in SBUF; phase B adds the odd k-tiles and writes out.
            with (
                tc.tile_pool(name="par", bufs=16) as par_pool,
                tc.tile_pool(name="oev", bufs=3) as oev_pool,
                tc.tile_pool(name="ps4", bufs=4, space="PSUM") as ps4,
            ):
                NCHUNK = D // CH
                MS = RPC // 128
                partial = {}
                for n in range(NCHUNK):
                    for m in range(MS):
                        ps_out = ps4.tile([128, CH], F32, tag="pout", name="ps_outA")
                        for i, kt in enumerate(range(0, KT, 2)):
                            nc.tensor.matmul(
                                ps_out[:],
                                attn_t[kt][:, m * 128 : (m + 1) * 128],
                                wts[(n, kt)][:],
                                start=(i == 0),
                                stop=(i == KT // 2 - 1),
                            )
                        par = par_pool.tile(
                            [128, CH], F32, tag="par", name=f"par{n}_{m}"
                        )
                        nc.vector.tensor_copy(par[:], ps_out[:])
                        partial[(n, m)] = par
                for n in range(NCHUNK):
                    for m in range(MS):
                        ps_out = ps4.tile([128, CH], F32, tag="pout", name="ps_outB")
                        for i, kt in enumerate(range(1, KT, 2)):
                            nc.tensor.matmul(
                                ps_out[:],
                                attn_t[kt][:, m * 128 : (m + 1) * 128],
                                wts[(n, kt)][:],
                                start=(i == 0),
                                stop=(i == KT // 2 - 1),
                            )
                        oev = oev_pool.tile([128, CH], F32, tag="oev")
                        nc.vector.tensor_tensor(
                            oev[:], ps_out[:], partial[(n, m)][:], ADD
                        )
                        nc.sync.dma_start(
                            out[m * 128 : (m + 1) * 128, n * CH : (n + 1) * CH],
                            oev[:],
                        )

        if dbg is not None:
            nc.sync.dma_start(dbg["dbg_q"], qT_sb[:])
            nc.sync.dma_start(dbg["dbg_k"], kT_sb[:])
            nc.sync.dma_start(dbg["dbg_v"], v_sb[:])
            for kt in range(KT):
                nc.sync.dma_start(dbg["dbg_attn"][:, kt, :], attn_t[kt][:])


_NC_CACHE = None


def _get_nc():
    global _NC_CACHE
    if _NC_CACHE is None:
        _NC_CACHE = build_nc()
    return _NC_CACHE


def _prep_inputs(x, freq_cos, freq_sin, wq, wk, wv, wo):
    bf = ml_dtypes.bfloat16
    x = np.asarray(x, np.float32).reshape(N, D)
    fc = np.asarray(freq_cos, np.float32)  # [S, 64]
    fs = np.asarray(freq_sin, np.float32)
    wq = np.asarray(wq, np.float32)
    wk = np.asarray(wk, np.float32)
    wv = np.asarray(wv, np.float32)
    wo = np.asarray(wo, np.float32)

    xT = np.ascontiguousarray(x.T).astype(bf)  # [D, N]
    woT = np.ascontiguousarray(wo.T).astype(bf)  # [D, D]

    # RoPE tables, expanded to the full head dim and tiled over batch.
    # fc2[d, b*S+i] = cos(freq[i, d//2]); fss carries sin with the sign of the
    # pair-swap term: -sin for even d, +sin for odd d.
    fc2 = np.tile(np.repeat(fc.T, 2, axis=0), (1, B)).astype(np.float32)
    sgn = np.where(np.arange(HD) % 2 == 0, -1.0, 1.0).astype(np.float32)[:, None]
    fss = (np.tile(np.repeat(fs.T, 2, axis=0), (1, B)) * sgn).astype(np.float32)
    fc2 = np.ascontiguousarray(fc2)
    fss = np.ascontiguousarray(fss)

    pswap = np.zeros((HD, HD), np.float32)
    pswap[np.arange(HD) ^ 1, np.arange(HD)] = 1.0
    pswap = pswap.astype(bf)

    # triangular 0/1 mask for the 128x128 diagonal corner: allow j <= i
    jp = np.arange(128)[:, None]
    ii = np.arange(128)[None, :]
    mask01 = (jp <= ii).astype(np.float32).astype(bf)

    scale = 1.0 / np.sqrt(HD)
    in_maps = []
    for c in range(W):
        rows = slice(c * DL, (c + 1) * DL)
        wqT = np.ascontiguousarray((wq[rows] * scale).T).astype(bf)
        wkT = np.ascontiguousarray(wk[rows].T).astype(bf)
        wvT = np.ascontiguousarray(wv[rows].T).astype(bf)
        in_maps.append(
            {
                "xT": xT,
                "wqT": wqT,
                "wkT": wkT,
                "wvT": wvT,
                "woT": woT,
                "fc2": fc2,
                "fss": fss,
                "pswap": pswap,
                "mask01": mask01,
            }
        )
    return in_maps


def kernel(x, freq_cos, freq_sin, wq, wk, wv, wo, _trace=False, _trace_kwargs=None):
    nc = _get_nc()
    in_maps = _prep_inputs(x, freq_cos, freq_sin, wq, wk, wv, wo)
    kwargs = {}
    if _trace:
        kwargs.update(trace=True, **(_trace_kwargs or {}))
    res = run_bass_kernel_spmd(nc, in_maps, core_ids=list(range(W)), **kwargs)
    kernel.last_result = res
    full = np.concatenate([res.results[c]["out"] for c in range(W)], axis=0)
    return full.reshape(B, S, D).astype(np.float32)


# revision 26
# speedup vs baseline: 1.4061x; 1.0003x over previous
"""Distributed Trainium2 Bass kernel for causal multi-head attention (RoPE).

Reference computation (B=2, S=2048, D=2048, H=16, hd=128):
    q/k/v = x @ w{q,k,v}.T ; rope(q, k) ; causal softmax attention ; out @ wo.T

Sharding over 8 NeuronCores (tensor-parallel over heads, then rows):
  - Each core owns 2 heads: computes its q/k/v projections (256 features),
    RoPE, and causal attention for those heads.
  - Attention outputs (normalized by the softmax denominator via a broadcast
    trick) are exchanged with one AllToAll per local head so each core ends
    up with ALL features for 1/8 of the token rows; the per-head split lets
    the first collective overlap the second head's attention compute.
  - Each core computes its 512 rows of the output projection; the host
    concatenates the 8 row-chunks.

Everything is computed in bf16 on the TensorEngine with f32 PSUM
accumulation; softmax runs without max-subtraction (scores are O(1) by
construction) with the causal mask applied as a 0/1 multiply after exp.

Layout tricks:
  - Activations live feature-major (xT, qT, kT) so matmul contractions are
    natural; v is produced token-major directly by swapping matmul operands.
  - Scores are computed transposed (sT[j, i]) so no on-chip transposes of the
    softmax matrix are needed; softmax sums over partitions use a ones-vector
    matmul; the per-token 1/Z is broadcast across partitions with a K=1
    matmul.
  - RoPE pair-swap (partition crossing) is done with a permutation-matrix
    matmul; cos/sin tables are pre-expanded on the host.
  - Causal structure: fully-masked j-blocks are skipped; on the 4 diagonal
    j-blocks of each 512-wide i-chunk only the live suffix of queries is
    computed, so just one triangular 128x128 corner needs the 0/1 mask.
  - The output projection runs in two phases (even k-tiles, then odd) with
    partial sums parked in SBUF, so a full pass of matmuls is available to
    overlap the second AllToAll.
"""

import numpy as np
import ml_dtypes

import concourse.mybir as mybir
import concourse.tile as tile
from concourse import bacc
from concourse.bass_utils import run_bass_kernel_spmd

# Problem constants (hardcoded per harness contract)
B, S, D, H = 2, 2048, 2048, 16
W = 8  # cores
N = B * S  # 4096 tokens
HD = D // H  # 128 head dim
HL = H // W  # 2 heads per core
DL = HL * HD  # 256 features per core
CH = 512  # token chunk
NCH = N // CH  # 8 chunks
KT = D // 128  # 16 contraction tiles
RPC = N // W  # 512 rows per core for the output projection
NVB = N // 128  # 32 v token-blocks
SB = S // CH  # 4 i-chunks per batch

F32 = mybir.dt.float32
BF16 = mybir.dt.bfloat16
MUL = mybir.AluOpType.mult
ADD = mybir.AluOpType.add


def build_nc(dumps=False):
    nc = bacc.Bacc("TRN2", target_bir_lowering=False, debug=False, num_devices=W)

    xT = nc.dram_tensor("xT", [D, N], BF16, kind="ExternalInput").ap()
    wqT = nc.dram_tensor("wqT", [D, DL], BF16, kind="ExternalInput").ap()
    wkT = nc.dram_tensor("wkT", [D, DL], BF16, kind="ExternalInput").ap()
    wvT = nc.dram_tensor("wvT", [D, DL], BF16, kind="ExternalInput").ap()
    woT = nc.dram_tensor("woT", [D, D], BF16, kind="ExternalInput").ap()
    fc2 = nc.dram_tensor("fc2", [HD, N], F32, kind="ExternalInput").ap()
    fss = nc.dram_tensor("fss", [HD, N], F32, kind="ExternalInput").ap()
    pswap = nc.dram_tensor("pswap", [HD, HD], BF16, kind="ExternalInput").ap()
    mask01 = nc.dram_tensor("mask01", [128, 128], BF16, kind="ExternalInput").ap()
    out = nc.dram_tensor("out", [RPC, D], F32, kind="ExternalOutput").ap()

    dbg = None
    if dumps:
        dbg = {
            "dbg_q": nc.dram_tensor("dbg_q", [128, HL, N], BF16, kind="ExternalOutput").ap(),
            "dbg_k": nc.dram_tensor("dbg_k", [128, HL, N], BF16, kind="ExternalOutput").ap(),
            "dbg_v": nc.dram_tensor("dbg_v", [128, NVB, DL], BF16, kind="ExternalOutput").ap(),
            "dbg_attn": nc.dram_tensor("dbg_attn", [128, KT, CH], BF16, kind="ExternalOutput").ap(),
        }

    with tile.TileContext(nc) as tc:
        _body(tc, xT, wqT, wkT, wvT, woT, fc2, fss, pswap, mask01, out, dbg)

    nc.compile()
    return nc


def _body(tc, xT, wqT, wkT, wvT, woT, fc2, fss, pswap, mask01, out, dbg=None):
    nc = tc.nc
    EXP = mybir.ActivationFunctionType.Exp

    with (
        tc.tile_pool(name="const", bufs=1) as const,
        tc.tile_pool(name="dram", bufs=1, space="DRAM") as dram,
    ):
        # ---- persistent SBUF state (weights first: stage 1 needs them now) ----
        # weights as half-tiles (kt 0-7 / 8-15) so the first matmuls only wait
        # on the first half-load
        wq_sb, wk_sb, wv_sb = {}, {}, {}
        for half in range(2):
            ksl = slice(half * 8 * 128, (half + 1) * 8 * 128)
            for d, src, nm in ((wq_sb, wqT, "q"), (wk_sb, wkT, "k"), (wv_sb, wvT, "v")):
                t = const.tile([128, 8, DL], BF16, name=f"w{nm}_h{half}")
                nc.sync.dma_start(t[:], src[ksl, :].rearrange("(kt p) m -> p kt m", p=128))
                d[half] = t
        pswap_sb = const.tile([128, 128], BF16)
        nc.sync.dma_start(pswap_sb[:], pswap)
        mask_sb = const.tile([128, 128], BF16)
        ones_col = const.tile([128, 1], BF16)
        nc.vector.memset(ones_col[:], 1.0)
        ones_row = const.tile([1, 128], BF16)
        nc.vector.memset(ones_row[:], 1.0)

        qT_sb = const.tile([128, HL, N], BF16)  # feature-major q (post-rope)
        kT_sb = const.tile([128, HL, N], BF16)
        v_sb = const.tile([128, NVB, DL], BF16)  # token-major v
        # post-A2A row tiles, feature-major; one tile per k-tile so phase-A
        # matmuls only depend on the first AllToAll's DMAs
        attn_t = [
            const.tile([128, CH], BF16, name=f"attn_t{kt}") for kt in range(KT)
        ]

        # per-head A2A buffers (shard s of head h = oT for rows [512s, 512s+512))
        a2a_in = [dram.tile([W, HD, CH], BF16, name=f"a2a_in{h}") for h in range(HL)]
        a2a_out = [dram.tile([W, HD, CH], BF16, name=f"a2a_out{h}") for h in range(HL)]

        # ================= stage 1: q/k/v projections + RoPE =================
        # K-contiguous per output tensor: all q matmuls for a chunk, then all
        # k, then all v (x tiles stay cached in SBUF).  Each tensor's PSUM
        # eviction then overlaps the next tensor's matmul phase, so chunk
        # boundaries don't stall the TensorEngine.
        with (
            tc.tile_pool(name="xin", bufs=24) as xin_pool,
            tc.tile_pool(name="ev", bufs=4) as ev_pool,
            tc.tile_pool(name="frq", bufs=1) as frq_pool,
            tc.tile_pool(name="ps1", bufs=1, space="PSUM") as ps1,
        ):
            fc2_sb = frq_pool.tile([128, N], F32)
            fss_sb = frq_pool.tile([128, N], F32)

            def rope_evict(ps_t, sub, dst, tok):
                tmp = ev_pool.tile([128, CH], BF16, tag="tmp")
                nc.scalar.copy(tmp[:], ps_t[:])
                ps_sw = ps1.tile([128, CH], F32, tag=f"psw{sub}", name="ps_sw")
                nc.tensor.matmul(ps_sw[:], pswap_sb[:], tmp[:], start=True, stop=True)
                t1 = ev_pool.tile([128, CH], F32, tag="t1")
                t2 = ev_pool.tile([128, CH], F32, tag="t2")
                nc.vector.tensor_tensor(t1[:], ps_t[:], fc2_sb[:, tok], MUL)
                nc.vector.tensor_tensor(t2[:], ps_sw[:], fss_sb[:, tok], MUL)
                nc.vector.tensor_tensor(dst[:, sub, tok], t1[:], t2[:], ADD)

            for ch in range(NCH):
                tok = slice(ch * CH, (ch + 1) * CH)
                xts = []
                for kt in range(KT):
                    xt = xin_pool.tile([128, CH], BF16, tag="xt", name=f"xt{kt}")
                    nc.sync.dma_start(xt[:], xT[kt * 128 : (kt + 1) * 128, tok])
                    xts.append(xt)
                if ch == 0:
                    # issued after chunk-0 loads so they don't delay the first
                    # matmuls; only needed by the rope eviction below
                    nc.sync.dma_start(fc2_sb[:], fc2)
                    nc.sync.dma_start(fss_sb[:], fss)
                    nc.sync.dma_start(mask_sb[:], mask01)
                ps_q = [
                    ps1.tile([128, CH], F32, tag=f"pq{s}", name=f"ps_q{s}")
                    for s in range(2)
                ]
                ps_k = [
                    ps1.tile([128, CH], F32, tag=f"pk{s}", name=f"ps_k{s}")
                    for s in range(2)
                ]
                ps_v = [
                    ps1.tile([128, 2, 256], F32, tag=f"pv{s}", name=f"ps_v{s}")
                    for s in range(2)
                ]
                for kt in range(KT):
                    st, sp = kt == 0, kt == KT - 1
                    for sub in range(2):
                        fsl = slice(sub * 128, (sub + 1) * 128)
                        nc.tensor.matmul(
                            ps_q[sub][:], wq_sb[kt // 8][:, kt % 8, fsl], xts[kt][:],
                            start=st, stop=sp,
                        )
                for sub in range(2):
                    rope_evict(ps_q[sub], sub, qT_sb, tok)
                for kt in range(KT):
                    st, sp = kt == 0, kt == KT - 1
                    for sub in range(2):
                        fsl = slice(sub * 128, (sub + 1) * 128)
                        nc.tensor.matmul(
                            ps_k[sub][:], wk_sb[kt // 8][:, kt % 8, fsl], xts[kt][:],
                            start=st, stop=sp,
                        )
                for sub in range(2):
                    rope_evict(ps_k[sub], sub, kT_sb, tok)
                for kt in range(KT):
                    st, sp = kt == 0, kt == KT - 1
                    for t in range(4):
                        # start=True zeroes the whole 2KB PSUM bank, so only
                        # the bank's first slice may set it (kt==0, even t)
                        nc.tensor.matmul(
                            ps_v[t // 2][:, t % 2, :],
                            xts[kt][:, t * 128 : (t + 1) * 128],
                            wv_sb[kt // 8][:, kt % 8, :],
                            start=(st and t % 2 == 0),
                            stop=sp,
                        )
                # evict v (token-major)
                for half in range(2):
                    nc.scalar.copy(
                        v_sb[:, ch * 4 + half * 2 : ch * 4 + half * 2 + 2, :],
                        ps_v[half][:],
                    )

        # preload ALL wo tiles on the gpsimd (SWDGE) queue so they stream in
        # during attention without delaying the sync-queue A2A staging writes
        with tc.tile_pool(name="wo", bufs=64) as wo_pool:
            wts = {}
            for n in range(D // CH):
                for kt in range(KT):
                    wt = wo_pool.tile([128, CH], BF16, tag="wo", name=f"wt{n}_{kt}")
                    nc.gpsimd.dma_start(
                        wt[:], woT[kt * 128 : (kt + 1) * 128, n * CH : (n + 1) * CH]
                    )
                    wts[(n, kt)] = wt

            # ================= stage 2: causal attention (head-outer) =========
            with (
                tc.tile_pool(name="pt", bufs=6) as pt_pool,
                tc.tile_pool(name="zv", bufs=2) as zv_pool,
                tc.tile_pool(name="ot", bufs=3) as ot_pool,
                tc.tile_pool(name="ps2", bufs=2, space="PSUM") as ps2,
            ):
                def emit_norm(h, b, ci, ps_o, zv):
                    # normalize by 1/Z (partition-sum via ones matmul,
                    # partition-broadcast via K=1 matmul)
                    zvb = pt_pool.tile([128, CH], BF16, tag="zvb")
                    nc.vector.tensor_copy(zvb[:], zv[:])
                    ps_z = ps2.tile([1, CH], F32, tag="pz", bufs=1)
                    nc.tensor.matmul(
                        ps_z[:], ones_col[:], zvb[:], start=True, stop=True
                    )
                    rz = ot_pool.tile([1, CH], F32, tag="rz")
                    nc.vector.reciprocal_approx_fast(rz[:], ps_z[:])
                    rzb = ot_pool.tile([1, CH], BF16, tag="rzb")
                    nc.vector.tensor_copy(rzb[:], rz[:])
                    ps_bc = ps2.tile([128, CH], F32, tag="pbc", bufs=1)
                    nc.tensor.matmul(
                        ps_bc[:], ones_row[:], rzb[:], start=True, stop=True
                    )
                    bc_sb = ot_pool.tile([128, CH], F32, tag="bc_sb")
                    nc.vector.tensor_copy(bc_sb[:], ps_bc[:])
                    otn = ot_pool.tile([128, CH], BF16, tag="otn")
                    nc.vector.tensor_tensor(otn[:], ps_o[:], bc_sb[:], MUL)
                    sh = b * SB + ci
                    nc.sync.dma_start(a2a_in[h][sh, :, :], otn[:])

                def emit_pv(jb, pt, off, wid, ps_o, njb, b, h):
                    vb = b * (S // 128) + jb
                    nc.tensor.matmul(
                        ps_o[:, off:],
                        v_sb[:, vb, h * 128 : (h + 1) * 128],
                        pt[:, :wid],
                        start=(jb == 0),
                        stop=(jb == njb - 1),
                    )

                for h in range(HL):
                    # software pipelines carried ACROSS group boundaries: pv
                    # matmuls run 2 j-blocks behind the score matmuls, and each
                    # group's Z-normalization chain is emitted a couple of
                    # score-steps into the next group, so the PE never waits on
                    # the exp/softmax chains
                    pend = []
                    pending_norm = None
                    norm_delay = 0
                    for b in range(B):
                        for ci in range(SB):
                            tok_i0 = b * S + ci * CH
                            ps_o = ps2.tile([128, CH], F32, tag="po", bufs=2)
                            zv = zv_pool.tile([128, CH], F32, tag="zv")
                            njb = 4 * ci + 4
                            for jb in range(njb):
                                tok_j = slice(b * S + jb * 128, b * S + (jb + 1) * 128)
                                r = jb - 4 * ci  # diag position (>=0 on diagonal)
                                off = 128 * r if r > 0 else 0  # live query suffix
                                wid = CH - off
                                ps_s = ps2.tile([128, CH], F32, tag="ps", bufs=4)
                                nc.tensor.matmul(
                                    ps_s[:, :wid],
                                    kT_sb[:, h, tok_j],
                                    qT_sb[:, h, tok_i0 + off : tok_i0 + CH],
                                    start=True,
                                    stop=True,
                                )
                                pt = pt_pool.tile([128, CH], BF16, tag="pt")
                                nc.scalar.activation(pt[:, :wid], ps_s[:, :wid], EXP)
                                if r >= 0:
                                    # triangular corner: queries [128r, 128r+128)
                                    nc.vector.tensor_tensor(
                                        pt[:, :128], pt[:, :128], mask_sb[:], MUL
                                    )
                                if jb == 0:
                                    nc.vector.tensor_copy(zv[:], pt[:])
                                else:
                                    nc.vector.tensor_tensor(
                                        zv[:, off:], zv[:, off:], pt[:, :wid], ADD
                                    )
                                pend.append((jb, pt, off, wid, ps_o, njb, b, h))
                                if len(pend) > 3:
                                    emit_pv(*pend.pop(0))
                                if pending_norm is not None:
                                    norm_delay -= 1
                                    if norm_delay <= 0:
                                        emit_norm(*pending_norm)
                                        pending_norm = None
                            if pending_norm is not None:
                                # short next group consumed fewer steps than
                                # norm_delay: flush before overwriting
                                emit_norm(*pending_norm)
                            pending_norm = (h, b, ci, ps_o, zv)
                            norm_delay = 5  # all of this group's pv matmuls
                            # have drained from `pend` after 2 more steps
                    while pend:
                        emit_pv(*pend.pop(0))
                    if pending_norm is not None:
                        emit_norm(*pending_norm)
                        pending_norm = None

                    # ---- per-head AllToAll: head 0's collective overlaps head
                    # 1's attention compute; head 1's overlaps phase A below
                    nc.gpsimd.collective_compute(
                        "AllToAll",
                        mybir.AluOpType.bypass,
                        replica_groups=[list(range(W))],
                        ins=[a2a_in[h].opt()],
                        outs=[a2a_out[h].opt()],
                    )
                    # pull this head's row tiles into SBUF right away
                    src = a2a_out[h][:].rearrange("w d c -> (w d) c")
                    for blk in range(W):
                        nc.sync.dma_start(
                            attn_t[2 * blk + h][:], src[blk * 128 : (blk + 1) * 128, :]
                        )

            # ============ stage 4: output projection for this core's rows =====
            # Two phases so ALL even-k (head-0) matmuls can run while the
            # second AllToAll is still in flight: phase A accumulates even
            # k-tiles for every (n, m) output group and parks the partial sums
            # in SBUF; phase B adds the odd k-tiles and writes out.
            with (
                tc.tile_pool(name="par", bufs=16) as par_pool,
                tc.tile_pool(name="oev", bufs=3) as oev_pool,
                tc.tile_pool(name="ps4", bufs=4, space="PSUM") as ps4,
            ):
                NCHUNK = D // CH
                MS = RPC // 128
                partial = {}
                for n in range(NCHUNK):
                    for m in range(MS):
                        ps_out = ps4.tile([128, CH], F32, tag="pout", name="ps_outA")
                        for i, kt in enumerate(range(0, KT, 2)):
                            nc.tensor.matmul(
                                ps_out[:],
                                attn_t[kt][:, m * 128 : (m + 1) * 128],
                                wts[(n, kt)][:],
                                start=(i == 0),
                                stop=(i == KT // 2 - 1),
                            )
                        par = par_pool.tile(
                            [128, CH], F32, tag="par", name=f"par{n}_{m}"
                        )
                        nc.vector.tensor_copy(par[:], ps_out[:])
                        partial[(n, m)] = par
                for n in range(NCHUNK):
                    for m in range(MS):
                        ps_out = ps4.tile([128, CH], F32, tag="pout", name="ps_outB")
                        for i, kt in enumerate(range(1, KT, 2)):
                            nc.tensor.matmul(
                                ps_out[:],
                                attn_t[kt][:, m * 128 : (m + 1) * 128],
                                wts[(n, kt)][:],
                                start=(i == 0),
                                stop=(i == KT // 2 - 1),
                            )
                        oev = oev_pool.tile([128, CH], F32, tag="oev")
                        nc.vector.tensor_tensor(
                            oev[:], ps_out[:], partial[(n, m)][:], ADD
                        )
                        nc.sync.dma_start(
                            out[m * 128 : (m + 1) * 128, n * CH : (n + 1) * CH],
                            oev[:],
                        )

        if dbg is not None:
            nc.sync.dma_start(dbg["dbg_q"], qT_sb[:])
            nc.sync.dma_start(dbg["dbg_k"], kT_sb[:])
            nc.sync.dma_start(dbg["dbg_v"], v_sb[:])
            for kt in range(KT):
                nc.sync.dma_start(dbg["dbg_attn"][:, kt, :], attn_t[kt][:])


_NC_CACHE = None


def _get_nc():
    global _NC_CACHE
    if _NC_CACHE is None:
        _NC_CACHE = build_nc()
    return _NC_CACHE


def _prep_inputs(x, freq_cos, freq_sin, wq, wk, wv, wo):
    bf = ml_dtypes.bfloat16
    x = np.asarray(x, np.float32).reshape(N, D)
    fc = np.asarray(freq_cos, np.float32)  # [S, 64]
    fs = np.asarray(freq_sin, np.float32)
    wq = np.asarray(wq, np.float32)
    wk = np.asarray(wk, np.float32)
    wv = np.asarray(wv, np.float32)
    wo = np.asarray(wo, np.float32)

    xT = np.ascontiguousarray(x.T).astype(bf)  # [D, N]
    woT = np.ascontiguousarray(wo.T).astype(bf)  # [D, D]

    # RoPE tables, expanded to the full head dim and tiled over batch.
    # fc2[d, b*S+i] = cos(freq[i, d//2]); fss carries sin with the sign of the
    # pair-swap term: -sin for even d, +sin for odd d.
    fc2 = np.tile(np.repeat(fc.T, 2, axis=0), (1, B)).astype(np.float32)
    sgn = np.where(np.arange(HD) % 2 == 0, -1.0, 1.0).astype(np.float32)[:, None]
    fss = (np.tile(np.repeat(fs.T, 2, axis=0), (1, B)) * sgn).astype(np.float32)
    fc2 = np.ascontiguousarray(fc2)
    fss = np.ascontiguousarray(fss)

    pswap = np.zeros((HD, HD), np.float32)
    pswap[np.arange(HD) ^ 1, np.arange(HD)] = 1.0
    pswap = pswap.astype(bf)

    # triangular 0/1 mask for the 128x128 diagonal corner: allow j <= i
    jp = np.arange(128)[:, None]
    ii = np.arange(128)[None, :]
    mask01 = (jp <= ii).astype(np.float32).astype(bf)

    scale = 1.0 / np.sqrt(HD)
    in_maps = []
    for c in range(W):
        rows = slice(c * DL, (c + 1) * DL)
        wqT = np.ascontiguousarray((wq[rows] * scale).T).astype(bf)
        wkT = np.ascontiguousarray(wk[rows].T).astype(bf)
        wvT = np.ascontiguousarray(wv[rows].T).astype(bf)
        in_maps.append(
            {
                "xT": xT,
                "wqT": wqT,
                "wkT": wkT,
                "wvT": wvT,
                "woT": woT,
                "fc2": fc2,
                "fss": fss,
                "pswap": pswap,
                "mask01": mask01,
            }
        )
    return in_maps


def kernel(x, freq_cos, freq_sin, wq, wk, wv, wo, _trace=False, _trace_kwargs=None):
    nc = _get_nc()
    in_maps = _prep_inputs(x, freq_cos, freq_sin, wq, wk, wv, wo)
    kwargs = {}
    if _trace:
        kwargs.update(trace=True, **(_trace_kwargs or {}))
    res = run_bass_kernel_spmd(nc, in_maps, core_ids=list(range(W)), **kwargs)
    kernel.last_result = res
    full = np.concatenate([res.results[c]["out"] for c in range(W)], axis=0)
    return full.reshape(B, S, D).astype(np.float32)
